# revision 27
# baseline (speedup 1.0000x reference)
"""MLA decode kernel for Trainium2, data-parallel over batch across 8 NeuronCores.

Each core handles 4 batches (M = 16 query rows).  v2 design vs baseline:
  - Attention matmuls stream the fp8 cache as the MOVING operand (512-col
    matmuls, ~144 per batch) with tiny Q/P stationaries, instead of pushing
    the cache through LDWEIGHTS as 128x128 stationary tiles (6400 instrs).
    Scores come out as S[hs, t]; softmax runs on the free axis (exp via
    ScalarE with fused row-sum accum_out).
  - P is transposed for PV with DVE StreamTranspose 32x32 blocks written at
    partition bases chosen per t-block, assembling a true [t%128, tc, hs]
    stationary layout in 8 instructions per batch.
  - PV output [hs, c] is PE-transposed (4 tiles) back to [c, hs] for the
    v-projection; v-proj / wo keep the baseline weight-stationary form.
  - Projections accumulate directly in PSUM across all k-chunks: one
    leading start=True per bank, then start=False everywhere - per-element
    has_written bits give overwrite-on-first-touch / accumulate-after,
    so many accumulation groups share a bank (validated on HW).  This
    removes the DVE accumulation chains that serialized the old q path.
  - kv/pe caches fp8 (e3m4) in both layouts; weights bf16.  Cache tiles are
    half-batch grained and prefetch through deep pools from t=0, spread
    over the sync/scalar/gpsimd DMA queues by need-time.
Host prep does layout/dtype only (transposes, tiling, fp8 cast) - no math.
"""

import os
import sys

sys.path.insert(0, "/opt/trn_rl_repo")

import numpy as np
import ml_dtypes

import concourse.bass as bass
import concourse.bacc as bacc_mod
import concourse.mybir as mybir
from concourse.bass_utils import run_bass_kernel_spmd
from concourse.masks import make_identity
from concourse.tile import TileContext

BF16 = mybir.dt.bfloat16
F32 = mybir.dt.float32
E3 = mybir.dt.float8e3
NBF = ml_dtypes.bfloat16
NE3 = ml_dtypes.float8_e3m4

DIM = 2048
N_HEADS = 16
Q_LORA = 1536
KV_LORA = 512
QK_NOPE = 128
QK_ROPE = 64
V_DIM = 128
QK_HD = QK_NOPE + QK_ROPE  # 192
MAX_SEQ = 8192
BSZ = 32
SEQLEN = 4
START_POS = MAX_SEQ - SEQLEN
EPS = 1e-6
SCALE = QK_HD ** -0.5

N_CORES = 8
BPC = BSZ // N_CORES          # batches per core = 4
M = BPC * SEQLEN              # rows per core = 16 (b, s)
HS = N_HEADS * SEQLEN         # 64 score rows per batch (h, s)
KQ = DIM // 128               # 16 k-chunks of x
KB = Q_LORA // 128            # 12 k-chunks of q_lora
R2 = QK_ROPE // 2             # 32
NTG = 16                      # t-groups of 512 per batch
TC = MAX_SEQ // 128           # 64 t-chunks of 128 per batch
HT = MAX_SEQ // 2             # 4096, half-tile width

AF = mybir.ActivationFunctionType
ALU = mybir.AluOpType


def build_bass():
    nc = bacc_mod.Bacc(target_bir_lowering=False)

    # ---- DRAM inputs (per core) ----
    xT = nc.dram_tensor("xT", [128, KQ, M], BF16, kind="ExternalInput")
    wqa = nc.dram_tensor("wqa", [KQ, 128, KB, 128], BF16, kind="ExternalInput")
    wqbn = nc.dram_tensor("wqbn", [KB, 128, N_HEADS, 128], BF16, kind="ExternalInput")
    wqbp = nc.dram_tensor("wqbp", [KB, 128, N_HEADS, QK_ROPE], BF16, kind="ExternalInput")
    wkval = nc.dram_tensor("wkval", [128, KQ, 4, 128], BF16, kind="ExternalInput")
    wkvap = nc.dram_tensor("wkvap", [128, KQ, QK_ROPE], BF16, kind="ExternalInput")
    wkvbn = nc.dram_tensor("wkvbn", [4, 128, 4, 4, 128], BF16, kind="ExternalInput")
    wkvbv = nc.dram_tensor("wkvbv", [4, 128, 4, 4, 128], BF16, kind="ExternalInput")
    wo = nc.dram_tensor("wo", [4, 4, 128, 4, 512], BF16, kind="ExternalInput")
    normw = nc.dram_tensor("normw", [128, KB + 4, 1], F32, kind="ExternalInput")
    ropet = nc.dram_tensor("ropet", [R2, 544], F32, kind="ExternalInput")
    klat = nc.dram_tensor("klat", [BPC, 4, 128, MAX_SEQ], E3, kind="ExternalInput")
    peS = nc.dram_tensor("peS", [BPC, QK_ROPE, MAX_SEQ], E3, kind="ExternalInput")
    kvnP = nc.dram_tensor("kvnP", [BPC, 4, 128, 16, 512], E3, kind="ExternalInput")
    out = nc.dram_tensor("out", [M, DIM], F32, kind="ExternalOutput")

    import contextlib

    with TileContext(nc) as tc, contextlib.ExitStack() as _ctx:
            cpool = _ctx.enter_context(tc.tile_pool(name="const", bufs=1))
            apool = _ctx.enter_context(tc.tile_pool(name="acts", bufs=1))
            wqa_pool = _ctx.enter_context(tc.tile_pool(name="wqa_s", bufs=3))
            wkval_pool = _ctx.enter_context(tc.tile_pool(name="wkval_s", bufs=2))
            wqb_pool = _ctx.enter_context(tc.tile_pool(name="wqb_s", bufs=3))
            wqbp_pool = _ctx.enter_context(tc.tile_pool(name="wqbp_s", bufs=3))
            wkvbn_pool = _ctx.enter_context(tc.tile_pool(name="wkvbn_s", bufs=4))
            kl_pool = _ctx.enter_context(tc.tile_pool(name="kl_s", bufs=5))
            pe_pool = _ctx.enter_context(tc.tile_pool(name="pe_s", bufs=2))
            kv_pool = _ctx.enter_context(tc.tile_pool(name="kv_s", bufs=5))
            pall_pool = _ctx.enter_context(tc.tile_pool(name="pall_s", bufs=8))
            pt_pool = _ctx.enter_context(tc.tile_pool(name="pt_s", bufs=8))
            fin_pool = _ctx.enter_context(tc.tile_pool(name="fin_s", bufs=1))
            ps_proj = _ctx.enter_context(tc.tile_pool(name="ps_proj", bufs=2, space="PSUM"))
            ps_small = _ctx.enter_context(tc.tile_pool(name="ps_small", bufs=1, space="PSUM"))
            ps_sc = _ctx.enter_context(tc.tile_pool(name="ps_sc", bufs=4, space="PSUM"))
            ps_pv = _ctx.enter_context(tc.tile_pool(name="ps_pv", bufs=1, space="PSUM"))
            # ---------- constants (gpsimd engine ops first) ----------
            ident = cpool.tile([128, 128], BF16)
            make_identity(nc, ident)
            ones_cf = cpool.tile([128, 1], F32)
            nc.gpsimd.memset(ones_cf, 1.0)
            ones_rf = cpool.tile([1, 128], F32)
            nc.gpsimd.memset(ones_rf, 1.0)
            eps1 = cpool.tile([1, 1], F32)
            nc.gpsimd.memset(eps1, EPS)

            # ---------- cache tiles b0 (gpsimd queue, from t=0) ----------
            # kl_tiles[b][j][half]: [128, 4096]; pe_tiles[b][half]: [64, 4096]
            kl_tiles = {}
            pe_tiles = {}

            def req_cache_tiles(b, eng):
                kl_tiles[b] = []
                for j in range(4):
                    t = kl_pool.tile([128, MAX_SEQ], E3, tag="kl")
                    eng.dma_start(t, klat[b, j])
                    kl_tiles[b].append(t)
                t = pe_pool.tile([QK_ROPE, MAX_SEQ], E3, tag="pe")
                eng.dma_start(t, peS[b])
                pe_tiles[b] = t

            # ---------- small residents (merged DMAs, sync queue) ----------
            xT_sb = cpool.tile([128, KQ, M], BF16)
            nc.sync.dma_start(xT_sb, xT[:, :, :])
            norm_sb = cpool.tile([128, KB + 4, 1], F32)
            nc.sync.dma_start(norm_sb, normw[:, :, :])
            qnwT_sb = norm_sb[:, 0:KB, :]
            kvnwT_sb = norm_sb[:, KB:KB + 4, :]
            rope_sb = cpool.tile([R2, 544], F32)
            nc.sync.dma_start(rope_sb, ropet[:, :])
            cosq_sb = rope_sb[:, 0:256].rearrange("p (h m) -> p h m", h=N_HEADS)
            sinq_sb = rope_sb[:, 256:512].rearrange("p (h m) -> p h m", h=N_HEADS)
            cosk_sb = rope_sb[:, 512:528]
            sink_sb = rope_sb[:, 528:544]
            wkvap_sb = cpool.tile([128, KQ, QK_ROPE], BF16)
            nc.sync.dma_start(wkvap_sb, wkvap[:, :, :])
            # wkval on gpsimd ahead of the b0 cache tiles
            wkval_sb = []
            for cnk in range(4):
                t = wkval_pool.tile([128, 4, 4, 128], BF16, tag="wkval")
                nc.gpsimd.dma_start(t, wkval[:, cnk * 4:(cnk + 1) * 4, :, :])
                wkval_sb.append(t)

            # wkvbn early (gpsimd) - absorb needs it by ~55us
            wkvbn_sb = []
            for cnk in range(4):
                t = wkvbn_pool.tile([128, 4, 4, 128], BF16, tag="wkvbn")
                nc.gpsimd.dma_start(t, wkvbn[cnk])
                wkvbn_sb.append(t)

            req_cache_tiles(0, nc.gpsimd)
            # NOTE: inserts for b0/b2 are issued later (after the gpsimd
            # cache DMAs) - an early insert waits on the kv path and would
            # stall the whole gpsimd FIFO for ~40us.

            # ---------- q1 = (x @ wq_a)^T accumulated in PSUM ----------
            psq = ps_proj.tile([128, KB, M], F32, tag="pp")
            for k in range(KQ):
                wqa_sb = wqa_pool.tile([128, KB, 128], BF16, tag="wqa")
                (nc.scalar if k % 2 == 0 else nc.sync).dma_start(
                    wqa_sb, wqa[k, :, :, :])
                for ncol in range(KB):
                    nc.tensor.matmul(psq[:, ncol, :], wqa_sb[:, ncol, :],
                                     xT_sb[:, k, :],
                                     start=(k == 0 and ncol == 0),
                                     stop=(k == KQ - 1 and ncol == KB - 1),
                                     skip_group_check=True)

            # ---------- kv path in PSUM ----------
            psk = ps_proj.tile([128, 5, M], F32, tag="pp")
            first = True
            for k in range(KQ):
                for j in range(4):
                    nc.tensor.matmul(psk[:, j, :],
                                     wkval_sb[k // 4][:, k % 4, j, :],
                                     xT_sb[:, k, :],
                                     start=first, stop=False,
                                     skip_group_check=True)
                    first = False
            for k in range(KQ):
                nc.tensor.matmul(psk[0:QK_ROPE, 4, :], wkvap_sb[:, k, :],
                                 xT_sb[:, k, :],
                                 start=False, stop=(k == KQ - 1),
                                 skip_group_check=True)

            def rms_factor(ps_in, nchunks, nfeat, tag):
                """Per-column 1/rms broadcast [128, M] f32 from transposed psum."""
                sq = apool.tile([128, nchunks, M], F32, tag=f"rms_sq{tag}")
                nc.scalar.activation(out=sq, in_=ps_in, func=AF.Square)
                msq = ps_small.tile([1, nchunks * M], F32, tag="ps")
                nc.tensor.matmul(msq, ones_cf,
                                 sq.rearrange("p a m -> p (a m)"),
                                 start=True, stop=True)
                msq_sb = apool.tile([1, nchunks * M], F32, tag=f"rms_msq{tag}")
                nc.vector.tensor_copy(msq_sb, msq)
                msq_v = msq_sb.rearrange("p (a m) -> p a m", a=nchunks)
                prev = msq_v[:, 0, :]
                for a in range(1, nchunks):
                    red = apool.tile([1, M], F32, tag=f"rms_red{tag}{a % 2}")
                    nc.vector.tensor_tensor(red, prev, msq_v[:, a, :], op=ALU.add)
                    prev = red
                rstd = apool.tile([1, M], F32, tag=f"rms_rstd{tag}")
                nc.scalar.activation(
                    out=rstd, in_=prev,
                    func=AF.Sqrt, scale=1.0 / nfeat, bias=eps1)
                rinv = apool.tile([1, M], F32, tag=f"rms_rinv{tag}")
                nc.vector.reciprocal(rinv, rstd)
                bc_ps = ps_small.tile([128, M], F32, tag="ps")
                nc.tensor.matmul(bc_ps, ones_rf, rinv, start=True, stop=True)
                bc = apool.tile([128, M], F32, tag=f"rms_bc{tag}")
                nc.vector.tensor_copy(bc, bc_ps)
                return bc

            # kv norm + casts
            bckv = rms_factor(psk[:, 0:4, :], 4, KV_LORA, "kv")
            kvlatT8 = apool.tile([128, 4, M], E3)
            kvlatT_bf = apool.tile([128, 4, M], BF16)
            for j in range(4):
                nc.vector.scalar_tensor_tensor(
                    out=kvlatT_bf[:, j, :], in0=psk[:, j, :],
                    scalar=kvnwT_sb[:, j, :], in1=bckv,
                    op0=ALU.mult, op1=ALU.mult)
            nc.vector.tensor_copy(kvlatT8, kvlatT_bf)
            # natural-layout fp8 copy for the PV-side insert
            kvlat8 = apool.tile([M, KV_LORA], E3)
            for j in range(4):
                ptb = ps_small.tile([M, 128], BF16, tag="ps")
                nc.tensor.transpose(ptb, kvlatT_bf[:, j, :], ident)
                nc.vector.tensor_copy(kvlat8[:, j * 128:(j + 1) * 128], ptb)

            # k_pe rope (even/odd-split rows already via host perm)
            kpeT8 = apool.tile([QK_ROPE, M], E3)
            t1k = apool.tile([R2, M], F32, tag="ropek1")
            t2k = apool.tile([R2, M], F32, tag="ropek2")
            nc.vector.tensor_tensor(t1k, psk[0:R2, 4, :], cosk_sb, op=ALU.mult)
            nc.vector.tensor_tensor(t2k, psk[R2:QK_ROPE, 4, :], sink_sb, op=ALU.mult)
            nc.vector.tensor_tensor(kpeT8[0:R2, :], t1k, t2k, op=ALU.subtract)
            t1k2 = apool.tile([R2, M], F32, tag="ropek1")
            t2k2 = apool.tile([R2, M], F32, tag="ropek2")
            nc.vector.tensor_tensor(t1k2, psk[0:R2, 4, :], sink_sb, op=ALU.mult)
            nc.vector.tensor_tensor(t2k2, psk[R2:QK_ROPE, 4, :], cosk_sb, op=ALU.mult)
            nc.vector.tensor_tensor(kpeT8[R2:QK_ROPE, :], t1k2, t2k2, op=ALU.add)

            # ---------- cache inserts (overwrite rows start_pos..) ----------
            def inserts_kl(b, eng):
                for j in range(4):
                    eng.dma_start(
                        kl_tiles[b][j][:, MAX_SEQ - SEQLEN:],
                        kvlatT8[:, j, b * SEQLEN:(b + 1) * SEQLEN])
                eng.dma_start(
                    pe_tiles[b][:, MAX_SEQ - SEQLEN:],
                    kpeT8[:, b * SEQLEN:(b + 1) * SEQLEN])

            # ---------- q1 rms + cast ----------
            bcq = rms_factor(psq, KB, Q_LORA, "q")
            q1nT = apool.tile([128, KB, M], BF16)
            for kc in range(KB):
                nc.vector.scalar_tensor_tensor(
                    out=q1nT[:, kc, :], in0=psq[:, kc, :],
                    scalar=qnwT_sb[:, kc, :], in1=bcq,
                    op0=ALU.mult, op1=ALU.mult)

            # ---------- q2: nope + pe in PSUM ----------
            psn = ps_proj.tile([128, N_HEADS, M], F32, tag="pp")
            psp = ps_proj.tile([128, N_HEADS, M], F32, tag="pp")
            fn = True
            fp = True
            for k in range(KB):
                wqbn_sb = wqb_pool.tile([128, N_HEADS, 128], BF16, tag="wqbn")
                (nc.scalar if k % 2 == 0 else nc.sync).dma_start(
                    wqbn_sb, wqbn[k, :, :, :])
                wqbp_sb = wqbp_pool.tile([128, N_HEADS, QK_ROPE], BF16, tag="wqbp")
                (nc.sync if k % 2 == 0 else nc.scalar).dma_start(
                    wqbp_sb, wqbp[k, :, :, :])
                for h in range(N_HEADS):
                    nc.tensor.matmul(psn[:, h, :], wqbn_sb[:, h, :],
                                     q1nT[:, k, :],
                                     start=fn,
                                     stop=(k == KB - 1 and h == N_HEADS - 1),
                                     skip_group_check=True)
                    fn = False
                for h in range(N_HEADS):
                    nc.tensor.matmul(psp[0:QK_ROPE, h, :], wqbp_sb[:, h, :],
                                     q1nT[:, k, :],
                                     start=fp,
                                     stop=(k == KB - 1 and h == N_HEADS - 1),
                                     skip_group_check=True)
                    fp = False

            # ---------- remaining cache tiles + inserts ----------
            # Queue choice avoids FIFO deadlocks: pool-gated DMA issues must
            # never sit ahead (same engine FIFO) of instructions attention
            # needs.  kvn b0 / wo n0-n1 on scalar are gate-free; gated kvn
            # b1 goes to sync (only kl b3/outs behind it), kvn b2/b3 and wo
            # n2-n3 to gpsimd (nothing early behind them).
            kvn_tiles = {}

            def req_kvn(b, eng):
                kvn_tiles[b] = []
                for gg in range(4):
                    t = kv_pool.tile([128, 16, 512], E3, tag="kvn")
                    eng.dma_start(t, kvnP[b, gg])
                    kvn_tiles[b].append(t)
                eng.dma_start(
                    kvn_tiles[b][3][128 - SEQLEN:, 15, :],
                    kvlat8[b * SEQLEN:(b + 1) * SEQLEN, :])

            req_cache_tiles(1, nc.sync)
            inserts_kl(1, nc.sync)
            req_kvn(0, nc.scalar)
            req_kvn(1, nc.gpsimd)
            req_cache_tiles(2, nc.gpsimd)
            inserts_kl(0, nc.gpsimd)
            inserts_kl(2, nc.gpsimd)
            req_kvn(2, nc.scalar)
            req_cache_tiles(3, nc.sync)
            inserts_kl(3, nc.sync)
            req_kvn(3, nc.gpsimd)

            # wkvbv through the kl ring: slots free as scores b2/b3 finish
            wkvbv_sb = []
            for cnk in range(4):
                t = kl_pool.tile([128, 4, 4, 128], BF16, tag="kl")
                nc.scalar.dma_start(t, wkvbv[cnk])
                wkvbv_sb.append(t)

            # wo through the kvn ring: slots free as PV consumes cache tiles,
            # so wo streams just-in-time during late attention
            wo_sb = {}
            for n in range(4):
                for kk in range(4):
                    wot = kv_pool.tile([128, 4, 512], BF16, tag="kvn")
                    eng = nc.scalar if n < 2 else nc.gpsimd
                    eng.dma_start(wot, wo[n, kk])
                    wo_sb[(n, kk)] = wot

            # ---------- q rope + absorb -> QT ----------
            QT = apool.tile([128, 5, BPC, HS], BF16)
            t1q = apool.tile([R2, N_HEADS, M], F32, tag="ropeq1")
            t2q = apool.tile([R2, N_HEADS, M], F32, tag="ropeq2")
            nc.vector.tensor_tensor(t1q, psp[0:R2, :, :], cosq_sb, op=ALU.mult)
            nc.vector.tensor_tensor(t2q, psp[R2:QK_ROPE, :, :], sinq_sb, op=ALU.mult)
            for b in range(BPC):
                nc.vector.tensor_tensor(
                    QT[0:R2, 4, b, :].rearrange("p (h s) -> p h s", h=N_HEADS),
                    t1q[:, :, b * SEQLEN:(b + 1) * SEQLEN],
                    t2q[:, :, b * SEQLEN:(b + 1) * SEQLEN],
                    op=ALU.subtract)
            t3q = apool.tile([R2, N_HEADS, M], F32, tag="ropeq1")
            t4q = apool.tile([R2, N_HEADS, M], F32, tag="ropeq2")
            nc.vector.tensor_tensor(t3q, psp[0:R2, :, :], sinq_sb, op=ALU.mult)
            nc.vector.tensor_tensor(t4q, psp[R2:QK_ROPE, :, :], cosq_sb, op=ALU.mult)
            for b in range(BPC):
                nc.vector.tensor_tensor(
                    QT[R2:QK_ROPE, 4, b, :].rearrange("p (h s) -> p h s", h=N_HEADS),
                    t3q[:, :, b * SEQLEN:(b + 1) * SEQLEN],
                    t4q[:, :, b * SEQLEN:(b + 1) * SEQLEN],
                    op=ALU.add)

            qnT_sb = apool.tile([128, N_HEADS, M], BF16)
            nc.vector.tensor_copy(qnT_sb, psn)

            # absorb: QT[:, cc, b, h*4+s] = sum_d wkvbn[h][d, c] * qnope[m, h, d]
            for h in range(N_HEADS):
                pa4 = ps_proj.tile([128, 4, M], F32, tag="pp")
                for cc in range(4):
                    nc.tensor.matmul(pa4[:, cc, :],
                                     wkvbn_sb[h // 4][:, h % 4, cc, :],
                                     qnT_sb[:, h, :], start=(cc == 0), stop=(cc == 3))
                nc.vector.tensor_copy(
                    QT[:, 0:4, :, h * SEQLEN:(h + 1) * SEQLEN],
                    pa4.rearrange("p cc (b s) -> p cc b s", b=BPC))

            # ---------- attention ----------
            # Scores for batch b+1 are emitted BEFORE PV of batch b, so the
            # PE never stalls waiting for b's kvn tiles (they get a full
            # extra batch-window to arrive) and the DVE P-transposes of b
            # overlap the next batch's score matmuls.
            outT = apool.tile([128, 4, N_HEADS, M], BF16)
            PTs = {}
            rinvs = {}

            def scores_phase(b):
                # per-quarter P and PT tiles: fine-grained deps let each
                # transpose start after its own 4 exps, and PV start after
                # the first quarter's transposes instead of all of them
                PTs[b] = []
                ssum_t = apool.tile([HS, NTG], F32, tag=f"ssum{b % 2}")

                def do_quarter(qq):
                    # two tc-sub-halves of the quarter packed on partitions
                    # 0-63 / 64-127 to keep the tile 128-partition dense
                    P_q = pall_pool.tile([128, 4, 8, 32], BF16, tag="pall")
                    PT_q = pt_pool.tile([128, 16, HS], BF16, tag="pt")
                    PTs[b].append(PT_q)
                    for tg in range(qq * 4, qq * 4 + 4):
                        sp = ps_sc.tile([128, 512], F32, tag="sp")
                        for j in range(4):
                            nc.tensor.matmul(
                                sp[0:HS, :], QT[:, j, b, :],
                                kl_tiles[b][j][:, tg * 512:(tg + 1) * 512],
                                start=(j == 0), stop=False,
                                skip_group_check=True)
                        nc.tensor.matmul(
                            sp[0:HS, :], QT[0:QK_ROPE, 4, b, :],
                            pe_tiles[b][:, tg * 512:(tg + 1) * 512],
                            start=False, stop=True,
                            skip_group_check=True)
                        tl = tg % 4
                        sub, tloc = divmod(tl, 2)
                        nc.scalar.activation(
                            out=P_q[64 * sub:64 * sub + HS, :,
                                    4 * tloc:4 * tloc + 4, :]
                                .rearrange("p r q bb -> p q r bb"),
                            in_=sp[0:HS, :],
                            func=AF.Exp, scale=SCALE,
                            accum_out=ssum_t[:, tg:tg + 1])
                    for r in range(4):
                        for i in range(2):
                            for sub in range(2):
                                nc.vector.transpose(
                                    PT_q[32 * r:32 * (r + 1),
                                         8 * sub:8 * sub + 8,
                                         32 * i:32 * (i + 1)],
                                    P_q[64 * sub + 32 * i:64 * sub + 32 * i + 32,
                                        r, :, :])

                for qq in range(4):
                    do_quarter(qq)

                # row sums -> 1/sum (DVE)
                s8 = apool.tile([HS, 8], F32, tag=f"att_s8{b % 2}")
                nc.vector.tensor_tensor(s8, ssum_t[:, 0:8], ssum_t[:, 8:16],
                                        op=ALU.add)
                s4 = apool.tile([HS, 4], F32, tag=f"att_s4{b % 2}")
                nc.vector.tensor_tensor(s4, s8[:, 0:4], s8[:, 4:8], op=ALU.add)
                s2 = apool.tile([HS, 2], F32, tag=f"att_s2{b % 2}")
                nc.vector.tensor_tensor(s2, s4[:, 0:2], s4[:, 2:4], op=ALU.add)
                ssum = apool.tile([HS, 1], F32, tag=f"att_ssum{b % 2}")
                nc.vector.tensor_tensor(ssum, s2[:, 0:1], s2[:, 1:2], op=ALU.add)
                rinv = apool.tile([HS, 1], F32, tag=f"att_rinv{b % 2}")
                nc.vector.reciprocal(rinv, ssum)
                rinvs[b] = rinv

            def pv_phase(b):
                po = ps_pv.tile([128, 512], F32, tag="po")
                for tci in range(TC):
                    nc.tensor.matmul(po[0:HS, :],
                                     PTs[b][tci // 16][:, tci % 16, :],
                                     kvn_tiles[b][tci // 16][:, tci % 16, :],
                                     start=(tci == 0), stop=(tci == TC - 1),
                                     skip_group_check=True)
                po_sb = apool.tile([HS, 512], BF16, tag=f"po_sb{b % 2}")
                nc.vector.tensor_scalar(out=po_sb, in0=po[0:HS, :],
                                        scalar1=rinvs[b],
                                        scalar2=None, op0=ALU.mult)
                psT = ps_small.tile([128, 4, HS], BF16, tag="ps")
                for cc in range(4):
                    nc.tensor.transpose(psT[:, cc, :],
                                        po_sb[:, cc * 128:(cc + 1) * 128],
                                        ident[0:HS, 0:HS])
                nc.vector.tensor_copy(
                    outT[:, :, :, b * SEQLEN:(b + 1) * SEQLEN],
                    psT.rearrange("p cc (h s) -> p cc h s", h=N_HEADS))

            for b in range(BPC):
                scores_phase(b)
                pv_phase(b)

            # ---------- v-proj: o2T[d, h, m] ----------
            o2T = apool.tile([128, N_HEADS, M], BF16)
            for h in range(N_HEADS):
                pv = ps_proj.tile([128, M], F32, tag="pp")
                for cc in range(4):
                    nc.tensor.matmul(pv, wkvbv_sb[h // 4][:, h % 4, cc, :],
                                     outT[:, cc, h, :],
                                     start=(cc == 0), stop=(cc == 3))
                nc.vector.tensor_copy(o2T[:, h, :], pv)

            # ---------- final: out = o2 @ wo ----------
            for n in range(4):
                pf = ps_proj.tile([M, 512], F32, tag="pp")
                for kk in range(4):
                    for j in range(4):
                        k = kk * 4 + j
                        nc.tensor.matmul(pf, o2T[:, k, :], wo_sb[(n, kk)][:, j, :],
                                         start=(k == 0), stop=(k == KQ - 1))
                fin = fin_pool.tile([M, 512], F32, tag="fin")
                nc.vector.tensor_copy(fin, pf)
                nc.sync.dma_start(out[:, n * 512:(n + 1) * 512], fin)

    nc.compile()
    return nc


_NC_CACHE = {}

# even/odd split permutation for rope dims: rows 0..31 = even pairs, 32..63 = odd
_PERM = np.concatenate([np.arange(0, QK_ROPE, 2), np.arange(1, QK_ROPE, 2)])


def prep_in_maps(x, wq_a, q_norm_w, wq_b, wkv_a, kv_norm_w, wkv_b, wo,
                 kv_cache, pe_cache, freqs_cos, freqs_sin, start_pos):
    assert int(start_pos) == START_POS
    bf = lambda a: np.ascontiguousarray(np.asarray(a, dtype=np.float32), dtype=NBF)
    f32 = lambda a: np.ascontiguousarray(np.asarray(a), dtype=np.float32)
    c = np.ascontiguousarray

    x = f32(x)
    wq_a = f32(wq_a); wq_b = f32(wq_b); wkv_a = f32(wkv_a)
    wkv_b_r = f32(wkv_b).reshape(N_HEADS, QK_NOPE + V_DIM, KV_LORA)
    wo_f = f32(wo)

    # --- weights, transposed/tiled layouts (bf16) ---
    wqa_t = bf(wq_a.reshape(KQ, 128, KB, 128))                    # [k,p,nc,d]
    wqb_r = wq_b.reshape(KB, 128, N_HEADS, QK_HD)                 # [k,p,h,d]
    wqbn_t = bf(wqb_r[:, :, :, :QK_NOPE])
    wqbp_t = bf(wqb_r[:, :, :, QK_NOPE:][:, :, :, _PERM])         # rope perm
    wkva_r = wkv_a.reshape(KQ, 128, KV_LORA + QK_ROPE)
    wkval_t = bf(wkva_r[:, :, :KV_LORA].reshape(KQ, 128, 4, 128)
                 .transpose(1, 0, 2, 3))                          # [p,k,j,d]
    wkvap_t = bf(wkva_r[:, :, KV_LORA:][:, :, _PERM].transpose(1, 0, 2))
    # absorb weights [d, h, c] -> chunks of 4 heads: [4, 128, 4h, 4cc, 128]
    wkvbn_t = bf(wkv_b_r[:, :QK_NOPE, :].transpose(1, 0, 2)       # [d, h, c]
                 .reshape(128, 4, 4, 4, 128).transpose(1, 0, 2, 3, 4))
    # v-proj weights [c, h, d] -> [c_in_chunk, h, cc, d] -> 4-head chunks
    wkvbv_t = bf(wkv_b_r[:, QK_NOPE:, :].transpose(2, 0, 1)
                 .reshape(4, 128, N_HEADS, V_DIM).transpose(1, 2, 0, 3)
                 .reshape(128, 4, 4, 4, V_DIM).transpose(1, 0, 2, 3, 4))
    # [n, kk, p, j, cc]: wo_t[n,kk,p,j,cc] = wo[(kk*4+j)*128+p, n*512+cc]
    wo_t = bf(wo_f.reshape(4, 4, 128, 4, 512).transpose(3, 0, 2, 1, 4))

    qnwT = f32(q_norm_w).reshape(KB, 128, 1).transpose(1, 0, 2)
    kvnwT = f32(kv_norm_w).reshape(4, 128, 1).transpose(1, 0, 2)
    normw = c(np.concatenate([qnwT, kvnwT], axis=1))              # [128, 16, 1]

    # --- rope tables packed [32, 544]: cosq(256) sinq(256) cosk(16) sink(16)
    cos = f32(freqs_cos); sin = f32(freqs_sin)                    # [s=4, 32]
    cosT = np.tile(cos.T, (1, BPC))                               # [32, 16] m=(b,s)
    sinT = np.tile(sin.T, (1, BPC))
    cosq_t = np.repeat(cosT[:, None, :], N_HEADS, axis=1)         # [32, h, 16]
    sinq_t = np.repeat(sinT[:, None, :], N_HEADS, axis=1)
    ropet = c(np.concatenate(
        [cosq_t.reshape(R2, 256), sinq_t.reshape(R2, 256), cosT, sinT],
        axis=1))

    # --- fp8 caches in both layouts ---
    kv8 = np.asarray(kv_cache, dtype=np.float32).astype(NE3)      # [32, 8192, 512]
    pe8 = np.asarray(pe_cache, dtype=np.float32).astype(NE3)      # [32, 8192, 64]

    in_maps = []
    for ci in range(N_CORES):
        bs = slice(ci * BPC, (ci + 1) * BPC)
        kvb = kv8[bs]                                             # [4, 8192, 512]
        peb = pe8[bs]
        # scores-side: klat[b,j,p,t] = kv[b, t, j*128+p]
        klat_ = c(kvb.reshape(BPC, MAX_SEQ, 4, 128).transpose(0, 2, 3, 1))
        # peS[b,r,t] = pe[b, t, perm(r)]
        peS_ = c(peb[:, :, _PERM].transpose(0, 2, 1))
        # pv-side: kvnP[b,gg,p,i,c] = kv[b, gg*2048 + i*128 + p, c]
        kvnP_ = c(kvb.reshape(BPC, 4, 16, 128, KV_LORA).transpose(0, 1, 3, 2, 4))
        xc = bf(x[bs].reshape(M, DIM).T.reshape(KQ, 128, M).transpose(1, 0, 2))
        in_maps.append({
            "xT": xc,
            "wqa": wqa_t, "wqbn": wqbn_t, "wqbp": wqbp_t,
            "wkval": wkval_t, "wkvap": wkvap_t,
            "wkvbn": wkvbn_t, "wkvbv": wkvbv_t, "wo": wo_t,
            "normw": normw, "ropet": ropet,
            "klat": klat_, "peS": peS_, "kvnP": kvnP_,
        })
    return in_maps


def kernel(**inputs):
    in_maps = prep_in_maps(**inputs)

    if "nc" not in _NC_CACHE:
        _NC_CACHE["nc"] = build_bass()
    nc = _NC_CACHE["nc"]

    trace = os.environ.get("KERNEL_TRACE", "0") == "1"
    tmpdir = os.environ.get("KERNEL_TMPDIR") or None
    res = run_bass_kernel_spmd(
        nc, in_maps, core_ids=list(range(N_CORES)), trace=trace, tmpdir=tmpdir
    )
    if trace and res.exec_time_ns is not None:
        print(f"HW exec time: {res.exec_time_ns} ns")
        _NC_CACHE["last_exec_ns"] = res.exec_time_ns

    _NC_CACHE["res"] = res
    _NC_CACHE["results"] = res.results
    outs = [r["out"].reshape(BPC, SEQLEN, DIM) for r in res.results]
    return np.concatenate(outs, axis=0).astype(np.float32)


# revision 28
# speedup vs baseline: 1.0173x; 1.0173x over previous
"""MLA decode kernel for Trainium2, data-parallel over batch across 8 NeuronCores.

Each core handles 4 batches (M = 16 query rows).  v2 design vs baseline:
  - Attention matmuls stream the fp8 cache as the MOVING operand (512-col
    matmuls, ~144 per batch) with tiny Q/P stationaries, instead of pushing
    the cache through LDWEIGHTS as 128x128 stationary tiles (6400 instrs).
    Scores come out as S[hs, t]; softmax runs on the free axis (exp via
    ScalarE with fused row-sum accum_out).
  - P is transposed for PV with DVE StreamTranspose 32x32 blocks written at
    partition bases chosen per t-block, assembling a true [t%128, tc, hs]
    stationary layout in 8 instructions per batch.
  - PV output [hs, c] is PE-transposed (4 tiles) back to [c, hs] for the
    v-projection; v-proj / wo keep the baseline weight-stationary form.
  - Projections accumulate directly in PSUM across all k-chunks: one
    leading start=True per bank, then start=False everywhere - per-element
    has_written bits give overwrite-on-first-touch / accumulate-after,
    so many accumulation groups share a bank (validated on HW).  This
    removes the DVE accumulation chains that serialized the old q path.
  - kv/pe caches fp8 (e3m4) in both layouts; weights bf16.  Cache tiles are
    half-batch grained and prefetch through deep pools from t=0, spread
    over the sync/scalar/gpsimd DMA queues by need-time.
Host prep does layout/dtype only (transposes, tiling, fp8 cast) - no math.
"""

import os
import sys

sys.path.insert(0, "/opt/trn_rl_repo")

import numpy as np
import ml_dtypes

import concourse.bass as bass
import concourse.bacc as bacc_mod
import concourse.mybir as mybir
from concourse.bass_utils import run_bass_kernel_spmd
from concourse.masks import make_identity
from concourse.tile import TileContext

BF16 = mybir.dt.bfloat16
F32 = mybir.dt.float32
E3 = mybir.dt.float8e3
NBF = ml_dtypes.bfloat16
NE3 = ml_dtypes.float8_e3m4

DIM = 2048
N_HEADS = 16
Q_LORA = 1536
KV_LORA = 512
QK_NOPE = 128
QK_ROPE = 64
V_DIM = 128
QK_HD = QK_NOPE + QK_ROPE  # 192
MAX_SEQ = 8192
BSZ = 32
SEQLEN = 4
START_POS = MAX_SEQ - SEQLEN
EPS = 1e-6
SCALE = QK_HD ** -0.5

N_CORES = 8
BPC = BSZ // N_CORES          # batches per core = 4
M = BPC * SEQLEN              # rows per core = 16 (b, s)
HS = N_HEADS * SEQLEN         # 64 score rows per batch (h, s)
KQ = DIM // 128               # 16 k-chunks of x
KB = Q_LORA // 128            # 12 k-chunks of q_lora
R2 = QK_ROPE // 2             # 32
NTG = 16                      # t-groups of 512 per batch
TC = MAX_SEQ // 128           # 64 t-chunks of 128 per batch
HT = MAX_SEQ // 2             # 4096, half-tile width

AF = mybir.ActivationFunctionType
ALU = mybir.AluOpType


def build_bass():
    nc = bacc_mod.Bacc(target_bir_lowering=False)

    # ---- DRAM inputs (per core) ----
    xT = nc.dram_tensor("xT", [128, KQ, M], BF16, kind="ExternalInput")
    wqa = nc.dram_tensor("wqa", [KQ, 128, KB, 128], BF16, kind="ExternalInput")
    wqbn = nc.dram_tensor("wqbn", [KB, 128, N_HEADS, 128], BF16, kind="ExternalInput")
    wqbp = nc.dram_tensor("wqbp", [KB, 128, N_HEADS, QK_ROPE], BF16, kind="ExternalInput")
    wkval = nc.dram_tensor("wkval", [128, KQ, 4, 128], BF16, kind="ExternalInput")
    wkvap = nc.dram_tensor("wkvap", [128, KQ, QK_ROPE], BF16, kind="ExternalInput")
    wkvbn = nc.dram_tensor("wkvbn", [4, 128, 4, 4, 128], BF16, kind="ExternalInput")
    wkvbv = nc.dram_tensor("wkvbv", [4, 128, 4, 4, 128], BF16, kind="ExternalInput")
    wo = nc.dram_tensor("wo", [4, 4, 128, 4, 512], BF16, kind="ExternalInput")
    normw = nc.dram_tensor("normw", [128, KB + 4, 1], F32, kind="ExternalInput")
    ropet = nc.dram_tensor("ropet", [R2, 544], F32, kind="ExternalInput")
    klat = nc.dram_tensor("klat", [BPC, 4, 128, MAX_SEQ], E3, kind="ExternalInput")
    peS = nc.dram_tensor("peS", [BPC, QK_ROPE, MAX_SEQ], E3, kind="ExternalInput")
    kvnP = nc.dram_tensor("kvnP", [BPC, 4, 128, 16, 512], E3, kind="ExternalInput")
    out = nc.dram_tensor("out", [M, DIM], F32, kind="ExternalOutput")

    import contextlib

    with TileContext(nc) as tc, contextlib.ExitStack() as _ctx:
            cpool = _ctx.enter_context(tc.tile_pool(name="const", bufs=1))
            apool = _ctx.enter_context(tc.tile_pool(name="acts", bufs=1))
            wqa_pool = _ctx.enter_context(tc.tile_pool(name="wqa_s", bufs=3))
            wkval_pool = _ctx.enter_context(tc.tile_pool(name="wkval_s", bufs=2))
            wqb_pool = _ctx.enter_context(tc.tile_pool(name="wqb_s", bufs=3))
            wqbp_pool = _ctx.enter_context(tc.tile_pool(name="wqbp_s", bufs=3))
            wkvbn_pool = _ctx.enter_context(tc.tile_pool(name="wkvbn_s", bufs=4))
            kl_pool = _ctx.enter_context(tc.tile_pool(name="kl_s", bufs=5))
            pe_pool = _ctx.enter_context(tc.tile_pool(name="pe_s", bufs=2))
            kv_pool = _ctx.enter_context(tc.tile_pool(name="kv_s", bufs=5))
            pall_pool = _ctx.enter_context(tc.tile_pool(name="pall_s", bufs=2))
            pt_pool = _ctx.enter_context(tc.tile_pool(name="pt_s", bufs=2))
            fin_pool = _ctx.enter_context(tc.tile_pool(name="fin_s", bufs=1))
            ps_proj = _ctx.enter_context(tc.tile_pool(name="ps_proj", bufs=2, space="PSUM"))
            ps_small = _ctx.enter_context(tc.tile_pool(name="ps_small", bufs=1, space="PSUM"))
            ps_sc = _ctx.enter_context(tc.tile_pool(name="ps_sc", bufs=4, space="PSUM"))
            ps_pv = _ctx.enter_context(tc.tile_pool(name="ps_pv", bufs=1, space="PSUM"))
            # ---------- constants (gpsimd engine ops first) ----------
            ident = cpool.tile([128, 128], BF16)
            make_identity(nc, ident)
            ones_cf = cpool.tile([128, 1], F32)
            nc.gpsimd.memset(ones_cf, 1.0)
            ones_rf = cpool.tile([1, 128], F32)
            nc.gpsimd.memset(ones_rf, 1.0)
            eps1 = cpool.tile([1, 1], F32)
            nc.gpsimd.memset(eps1, EPS)

            # ---------- cache tiles b0 (gpsimd queue, from t=0) ----------
            # kl_tiles[b][j][half]: [128, 4096]; pe_tiles[b][half]: [64, 4096]
            kl_tiles = {}
            pe_tiles = {}

            def req_cache_tiles(b, eng):
                kl_tiles[b] = []
                for j in range(4):
                    t = kl_pool.tile([128, MAX_SEQ], E3, tag="kl")
                    eng.dma_start(t, klat[b, j])
                    kl_tiles[b].append(t)
                t = pe_pool.tile([QK_ROPE, MAX_SEQ], E3, tag="pe")
                eng.dma_start(t, peS[b])
                pe_tiles[b] = t

            # ---------- small residents (merged DMAs, sync queue) ----------
            xT_sb = cpool.tile([128, KQ, M], BF16)
            nc.sync.dma_start(xT_sb, xT[:, :, :])
            norm_sb = cpool.tile([128, KB + 4, 1], F32)
            nc.sync.dma_start(norm_sb, normw[:, :, :])
            qnwT_sb = norm_sb[:, 0:KB, :]
            kvnwT_sb = norm_sb[:, KB:KB + 4, :]
            rope_sb = cpool.tile([R2, 544], F32)
            nc.sync.dma_start(rope_sb, ropet[:, :])
            cosq_sb = rope_sb[:, 0:256].rearrange("p (h m) -> p h m", h=N_HEADS)
            sinq_sb = rope_sb[:, 256:512].rearrange("p (h m) -> p h m", h=N_HEADS)
            cosk_sb = rope_sb[:, 512:528]
            sink_sb = rope_sb[:, 528:544]
            wkvap_sb = cpool.tile([128, KQ, QK_ROPE], BF16)
            nc.sync.dma_start(wkvap_sb, wkvap[:, :, :])
            # wkval on gpsimd ahead of the b0 cache tiles
            wkval_sb = []
            for cnk in range(4):
                t = wkval_pool.tile([128, 4, 4, 128], BF16, tag="wkval")
                nc.gpsimd.dma_start(t, wkval[:, cnk * 4:(cnk + 1) * 4, :, :])
                wkval_sb.append(t)

            # wkvbn early (gpsimd) - absorb needs it by ~55us
            wkvbn_sb = []
            for cnk in range(4):
                t = wkvbn_pool.tile([128, 4, 4, 128], BF16, tag="wkvbn")
                nc.gpsimd.dma_start(t, wkvbn[cnk])
                wkvbn_sb.append(t)

            req_cache_tiles(0, nc.gpsimd)

            # ---------- q1 = (x @ wq_a)^T accumulated in PSUM ----------
            psq = ps_proj.tile([128, KB, M], F32, tag="pp")
            for k in range(KQ):
                wqa_sb = wqa_pool.tile([128, KB, 128], BF16, tag="wqa")
                (nc.scalar if k % 2 == 0 else nc.sync).dma_start(
                    wqa_sb, wqa[k, :, :, :])
                for ncol in range(KB):
                    nc.tensor.matmul(psq[:, ncol, :], wqa_sb[:, ncol, :],
                                     xT_sb[:, k, :],
                                     start=(k == 0 and ncol == 0),
                                     stop=(k == KQ - 1 and ncol == KB - 1),
                                     skip_group_check=True)

            # ---------- kv path in PSUM ----------
            psk = ps_proj.tile([128, 5, M], F32, tag="pp")
            first = True
            for k in range(KQ):
                for j in range(4):
                    nc.tensor.matmul(psk[:, j, :],
                                     wkval_sb[k // 4][:, k % 4, j, :],
                                     xT_sb[:, k, :],
                                     start=first, stop=False,
                                     skip_group_check=True)
                    first = False
            for k in range(KQ):
                nc.tensor.matmul(psk[0:QK_ROPE, 4, :], wkvap_sb[:, k, :],
                                 xT_sb[:, k, :],
                                 start=False, stop=(k == KQ - 1),
                                 skip_group_check=True)

            def rms_factor(ps_in, nchunks, nfeat, tag):
                """Per-column 1/rms broadcast [128, M] f32 from transposed psum."""
                sq = apool.tile([128, nchunks, M], F32, tag=f"rms_sq{tag}")
                nc.scalar.activation(out=sq, in_=ps_in, func=AF.Square)
                msq = ps_small.tile([1, nchunks * M], F32, tag="ps")
                nc.tensor.matmul(msq, ones_cf,
                                 sq.rearrange("p a m -> p (a m)"),
                                 start=True, stop=True)
                msq_sb = apool.tile([1, nchunks * M], F32, tag=f"rms_msq{tag}")
                nc.vector.tensor_copy(msq_sb, msq)
                msq_v = msq_sb.rearrange("p (a m) -> p a m", a=nchunks)
                prev = msq_v[:, 0, :]
                for a in range(1, nchunks):
                    red = apool.tile([1, M], F32, tag=f"rms_red{tag}{a % 2}")
                    nc.vector.tensor_tensor(red, prev, msq_v[:, a, :], op=ALU.add)
                    prev = red
                rstd = apool.tile([1, M], F32, tag=f"rms_rstd{tag}")
                nc.scalar.activation(
                    out=rstd, in_=prev,
                    func=AF.Sqrt, scale=1.0 / nfeat, bias=eps1)
                rinv = apool.tile([1, M], F32, tag=f"rms_rinv{tag}")
                nc.vector.reciprocal(rinv, rstd)
                bc_ps = ps_small.tile([128, M], F32, tag="ps")
                nc.tensor.matmul(bc_ps, ones_rf, rinv, start=True, stop=True)
                bc = apool.tile([128, M], F32, tag=f"rms_bc{tag}")
                nc.vector.tensor_copy(bc, bc_ps)
                return bc

            # kv norm + casts
            bckv = rms_factor(psk[:, 0:4, :], 4, KV_LORA, "kv")
            kvlatT8 = apool.tile([128, 4, M], E3)
            kvlatT_bf = apool.tile([128, 4, M], BF16)
            for j in range(4):
                nc.vector.scalar_tensor_tensor(
                    out=kvlatT_bf[:, j, :], in0=psk[:, j, :],
                    scalar=kvnwT_sb[:, j, :], in1=bckv,
                    op0=ALU.mult, op1=ALU.mult)
            nc.vector.tensor_copy(kvlatT8, kvlatT_bf)
            # natural-layout fp8 copy for the PV-side insert
            kvlat8 = apool.tile([M, KV_LORA], E3)
            for j in range(4):
                ptb = ps_small.tile([M, 128], BF16, tag="ps")
                nc.tensor.transpose(ptb, kvlatT_bf[:, j, :], ident)
                nc.vector.tensor_copy(kvlat8[:, j * 128:(j + 1) * 128], ptb)

            # k_pe rope (even/odd-split rows already via host perm)
            kpeT8 = apool.tile([QK_ROPE, M], E3)
            t1k = apool.tile([R2, M], F32, tag="ropek1")
            t2k = apool.tile([R2, M], F32, tag="ropek2")
            nc.vector.tensor_tensor(t1k, psk[0:R2, 4, :], cosk_sb, op=ALU.mult)
            nc.vector.tensor_tensor(t2k, psk[R2:QK_ROPE, 4, :], sink_sb, op=ALU.mult)
            nc.vector.tensor_tensor(kpeT8[0:R2, :], t1k, t2k, op=ALU.subtract)
            t1k2 = apool.tile([R2, M], F32, tag="ropek1")
            t2k2 = apool.tile([R2, M], F32, tag="ropek2")
            nc.vector.tensor_tensor(t1k2, psk[0:R2, 4, :], sink_sb, op=ALU.mult)
            nc.vector.tensor_tensor(t2k2, psk[R2:QK_ROPE, 4, :], cosk_sb, op=ALU.mult)
            nc.vector.tensor_tensor(kpeT8[R2:QK_ROPE, :], t1k2, t2k2, op=ALU.add)

            # ---------- cache inserts (overwrite rows start_pos..) ----------
            def inserts_kl(b, eng):
                for j in range(4):
                    eng.dma_start(
                        kl_tiles[b][j][:, MAX_SEQ - SEQLEN:],
                        kvlatT8[:, j, b * SEQLEN:(b + 1) * SEQLEN])
                eng.dma_start(
                    pe_tiles[b][:, MAX_SEQ - SEQLEN:],
                    kpeT8[:, b * SEQLEN:(b + 1) * SEQLEN])

            inserts_kl(0, nc.gpsimd)

            # ---------- q1 rms + cast ----------
            bcq = rms_factor(psq, KB, Q_LORA, "q")
            q1nT = apool.tile([128, KB, M], BF16)
            for kc in range(KB):
                nc.vector.scalar_tensor_tensor(
                    out=q1nT[:, kc, :], in0=psq[:, kc, :],
                    scalar=qnwT_sb[:, kc, :], in1=bcq,
                    op0=ALU.mult, op1=ALU.mult)

            # ---------- q2: nope + pe in PSUM ----------
            psn = ps_proj.tile([128, N_HEADS, M], F32, tag="pp")
            psp = ps_proj.tile([128, N_HEADS, M], F32, tag="pp")
            fn = True
            fp = True
            for k in range(KB):
                wqbn_sb = wqb_pool.tile([128, N_HEADS, 128], BF16, tag="wqbn")
                (nc.scalar if k % 2 == 0 else nc.sync).dma_start(
                    wqbn_sb, wqbn[k, :, :, :])
                wqbp_sb = wqbp_pool.tile([128, N_HEADS, QK_ROPE], BF16, tag="wqbp")
                (nc.sync if k % 2 == 0 else nc.scalar).dma_start(
                    wqbp_sb, wqbp[k, :, :, :])
                for h in range(N_HEADS):
                    nc.tensor.matmul(psn[:, h, :], wqbn_sb[:, h, :],
                                     q1nT[:, k, :],
                                     start=fn,
                                     stop=(k == KB - 1 and h == N_HEADS - 1),
                                     skip_group_check=True)
                    fn = False
                for h in range(N_HEADS):
                    nc.tensor.matmul(psp[0:QK_ROPE, h, :], wqbp_sb[:, h, :],
                                     q1nT[:, k, :],
                                     start=fp,
                                     stop=(k == KB - 1 and h == N_HEADS - 1),
                                     skip_group_check=True)
                    fp = False

            # ---------- remaining cache tiles + inserts ----------
            # Queue choice avoids FIFO deadlocks: pool-gated DMA issues must
            # never sit ahead (same engine FIFO) of instructions attention
            # needs.  kvn b0 / wo n0-n1 on scalar are gate-free; gated kvn
            # b1 goes to sync (only kl b3/outs behind it), kvn b2/b3 and wo
            # n2-n3 to gpsimd (nothing early behind them).
            kvn_tiles = {}

            def req_kvn(b, eng):
                kvn_tiles[b] = []
                for gg in range(4):
                    t = kv_pool.tile([128, 16, 512], E3, tag="kvn")
                    eng.dma_start(t, kvnP[b, gg])
                    kvn_tiles[b].append(t)
                eng.dma_start(
                    kvn_tiles[b][3][128 - SEQLEN:, 15, :],
                    kvlat8[b * SEQLEN:(b + 1) * SEQLEN, :])

            req_cache_tiles(1, nc.sync)
            inserts_kl(1, nc.sync)
            req_kvn(0, nc.scalar)
            req_kvn(1, nc.gpsimd)
            req_cache_tiles(2, nc.gpsimd)
            inserts_kl(2, nc.gpsimd)
            req_kvn(2, nc.scalar)
            req_cache_tiles(3, nc.sync)
            inserts_kl(3, nc.sync)
            req_kvn(3, nc.gpsimd)

            # wkvbv through the kl ring: slots free as scores b2/b3 finish
            wkvbv_sb = []
            for cnk in range(4):
                t = kl_pool.tile([128, 4, 4, 128], BF16, tag="kl")
                nc.scalar.dma_start(t, wkvbv[cnk])
                wkvbv_sb.append(t)

            # wo through the kvn ring: slots free as PV consumes cache tiles,
            # so wo streams just-in-time during late attention
            wo_sb = {}
            for n in range(4):
                for kk in range(4):
                    wot = kv_pool.tile([128, 4, 512], BF16, tag="kvn")
                    eng = nc.scalar if n < 2 else nc.gpsimd
                    eng.dma_start(wot, wo[n, kk])
                    wo_sb[(n, kk)] = wot

            # ---------- q rope + absorb -> QT ----------
            QT = apool.tile([128, 5, BPC, HS], BF16)
            t1q = apool.tile([R2, N_HEADS, M], F32, tag="ropeq1")
            t2q = apool.tile([R2, N_HEADS, M], F32, tag="ropeq2")
            nc.vector.tensor_tensor(t1q, psp[0:R2, :, :], cosq_sb, op=ALU.mult)
            nc.vector.tensor_tensor(t2q, psp[R2:QK_ROPE, :, :], sinq_sb, op=ALU.mult)
            for b in range(BPC):
                nc.vector.tensor_tensor(
                    QT[0:R2, 4, b, :].rearrange("p (h s) -> p h s", h=N_HEADS),
                    t1q[:, :, b * SEQLEN:(b + 1) * SEQLEN],
                    t2q[:, :, b * SEQLEN:(b + 1) * SEQLEN],
                    op=ALU.subtract)
            t3q = apool.tile([R2, N_HEADS, M], F32, tag="ropeq1")
            t4q = apool.tile([R2, N_HEADS, M], F32, tag="ropeq2")
            nc.vector.tensor_tensor(t3q, psp[0:R2, :, :], sinq_sb, op=ALU.mult)
            nc.vector.tensor_tensor(t4q, psp[R2:QK_ROPE, :, :], cosq_sb, op=ALU.mult)
            for b in range(BPC):
                nc.vector.tensor_tensor(
                    QT[R2:QK_ROPE, 4, b, :].rearrange("p (h s) -> p h s", h=N_HEADS),
                    t3q[:, :, b * SEQLEN:(b + 1) * SEQLEN],
                    t4q[:, :, b * SEQLEN:(b + 1) * SEQLEN],
                    op=ALU.add)

            qnT_sb = apool.tile([128, N_HEADS, M], BF16)
            nc.vector.tensor_copy(qnT_sb, psn)

            # absorb: QT[:, cc, b, h*4+s] = sum_d wkvbn[h][d, c] * qnope[m, h, d]
            for h in range(N_HEADS):
                pa4 = ps_small.tile([128, 4, M], F32, tag="ps")
                for cc in range(4):
                    nc.tensor.matmul(pa4[:, cc, :],
                                     wkvbn_sb[h // 4][:, h % 4, cc, :],
                                     qnT_sb[:, h, :], start=(cc == 0), stop=(cc == 3))
                nc.vector.tensor_copy(
                    QT[:, 0:4, :, h * SEQLEN:(h + 1) * SEQLEN],
                    pa4.rearrange("p cc (b s) -> p cc b s", b=BPC))

            # ---------- attention ----------
            # Scores for batch b+1 are emitted BEFORE PV of batch b, so the
            # PE never stalls waiting for b's kvn tiles (they get a full
            # extra batch-window to arrive) and the DVE P-transposes of b
            # overlap the next batch's score matmuls.
            outT = apool.tile([128, 4, N_HEADS, M], BF16)
            PTs = {}
            rinvs = {}

            def scores_phase(b):
                P_all = pall_pool.tile([128, 4, TC // 2, 32], BF16, tag="pall")
                PT = pt_pool.tile([128, TC, HS], BF16, tag="pt")
                PTs[b] = PT
                ssum_t = apool.tile([HS, NTG], F32, tag=f"ssum{b % 2}")

                def scores_quarter(qq):
                    for tg in range(qq * 4, qq * 4 + 4):
                        sp = ps_sc.tile([128, 512], F32, tag="sp")
                        for j in range(4):
                            nc.tensor.matmul(
                                sp[0:HS, :], QT[:, j, b, :],
                                kl_tiles[b][j][:, tg * 512:(tg + 1) * 512],
                                start=(j == 0), stop=False,
                                skip_group_check=True)
                        nc.tensor.matmul(
                            sp[0:HS, :], QT[0:QK_ROPE, 4, b, :],
                            pe_tiles[b][:, tg * 512:(tg + 1) * 512],
                            start=False, stop=True,
                            skip_group_check=True)
                        pb_ = 64 * (tg // 8)
                        tl = 4 * (tg % 8)
                        nc.scalar.activation(
                            out=P_all[pb_:pb_ + HS, :, tl:tl + 4, :]
                                .rearrange("p r q bb -> p q r bb"),
                            in_=sp[0:HS, :],
                            func=AF.Exp, scale=SCALE,
                            accum_out=ssum_t[:, tg:tg + 1])

                def transpose_quarter(qq):
                    half, sub = divmod(qq, 2)
                    for r in range(4):
                        for i in range(2):
                            pb_ = 64 * half + 32 * i
                            nc.vector.transpose(
                                PT[32 * r:32 * (r + 1), 16 * qq:16 * qq + 16,
                                   32 * i:32 * (i + 1)],
                                P_all[pb_:pb_ + 32, r,
                                      16 * sub:16 * sub + 16, :])

                for qq in range(4):
                    scores_quarter(qq)
                    transpose_quarter(qq)

                # row sums -> 1/sum (DVE)
                s8 = apool.tile([HS, 8], F32, tag=f"att_s8{b % 2}")
                nc.vector.tensor_tensor(s8, ssum_t[:, 0:8], ssum_t[:, 8:16],
                                        op=ALU.add)
                s4 = apool.tile([HS, 4], F32, tag=f"att_s4{b % 2}")
                nc.vector.tensor_tensor(s4, s8[:, 0:4], s8[:, 4:8], op=ALU.add)
                s2 = apool.tile([HS, 2], F32, tag=f"att_s2{b % 2}")
                nc.vector.tensor_tensor(s2, s4[:, 0:2], s4[:, 2:4], op=ALU.add)
                ssum = apool.tile([HS, 1], F32, tag=f"att_ssum{b % 2}")
                nc.vector.tensor_tensor(ssum, s2[:, 0:1], s2[:, 1:2], op=ALU.add)
                rinv = apool.tile([HS, 1], F32, tag=f"att_rinv{b % 2}")
                nc.vector.reciprocal(rinv, ssum)
                rinvs[b] = rinv

            def pv_phase(b):
                PT = PTs[b]
                po = ps_pv.tile([128, 512], F32, tag="po")
                for tci in range(TC):
                    nc.tensor.matmul(po[0:HS, :], PT[:, tci, :],
                                     kvn_tiles[b][tci // 16][:, tci % 16, :],
                                     start=(tci == 0), stop=(tci == TC - 1),
                                     skip_group_check=True)
                po_sb = apool.tile([HS, 512], BF16, tag=f"po_sb{b % 2}")
                nc.vector.tensor_scalar(out=po_sb, in0=po[0:HS, :],
                                        scalar1=rinvs[b],
                                        scalar2=None, op0=ALU.mult)
                psT = ps_small.tile([128, 4, HS], BF16, tag="ps")
                for cc in range(4):
                    nc.tensor.transpose(psT[:, cc, :],
                                        po_sb[:, cc * 128:(cc + 1) * 128],
                                        ident[0:HS, 0:HS])
                nc.vector.tensor_copy(
                    outT[:, :, :, b * SEQLEN:(b + 1) * SEQLEN],
                    psT.rearrange("p cc (h s) -> p cc h s", h=N_HEADS))

            for b in range(BPC):
                scores_phase(b)
                pv_phase(b)

            # ---------- v-proj: o2T[d, h, m] ----------
            o2T = apool.tile([128, N_HEADS, M], BF16)
            for h in range(N_HEADS):
                pv = ps_small.tile([128, M], F32, tag="ps")
                for cc in range(4):
                    nc.tensor.matmul(pv, wkvbv_sb[h // 4][:, h % 4, cc, :],
                                     outT[:, cc, h, :],
                                     start=(cc == 0), stop=(cc == 3))
                nc.vector.tensor_copy(o2T[:, h, :], pv)

            # ---------- final: out = o2 @ wo ----------
            for n in range(4):
                pf = ps_proj.tile([M, 512], F32, tag="pp")
                for kk in range(4):
                    for j in range(4):
                        k = kk * 4 + j
                        nc.tensor.matmul(pf, o2T[:, k, :], wo_sb[(n, kk)][:, j, :],
                                         start=(k == 0), stop=(k == KQ - 1))
                fin = fin_pool.tile([M, 512], F32, tag="fin")
                nc.vector.tensor_copy(fin, pf)
                nc.sync.dma_start(out[:, n * 512:(n + 1) * 512], fin)

    nc.compile()
    return nc


_NC_CACHE = {}

# even/odd split permutation for rope dims: rows 0..31 = even pairs, 32..63 = odd
_PERM = np.concatenate([np.arange(0, QK_ROPE, 2), np.arange(1, QK_ROPE, 2)])


def prep_in_maps(x, wq_a, q_norm_w, wq_b, wkv_a, kv_norm_w, wkv_b, wo,
                 kv_cache, pe_cache, freqs_cos, freqs_sin, start_pos):
    assert int(start_pos) == START_POS
    bf = lambda a: np.ascontiguousarray(np.asarray(a, dtype=np.float32), dtype=NBF)
    f32 = lambda a: np.ascontiguousarray(np.asarray(a), dtype=np.float32)
    c = np.ascontiguousarray

    x = f32(x)
    wq_a = f32(wq_a); wq_b = f32(wq_b); wkv_a = f32(wkv_a)
    wkv_b_r = f32(wkv_b).reshape(N_HEADS, QK_NOPE + V_DIM, KV_LORA)
    wo_f = f32(wo)

    # --- weights, transposed/tiled layouts (bf16) ---
    wqa_t = bf(wq_a.reshape(KQ, 128, KB, 128))                    # [k,p,nc,d]
    wqb_r = wq_b.reshape(KB, 128, N_HEADS, QK_HD)                 # [k,p,h,d]
    wqbn_t = bf(wqb_r[:, :, :, :QK_NOPE])
    wqbp_t = bf(wqb_r[:, :, :, QK_NOPE:][:, :, :, _PERM])         # rope perm
    wkva_r = wkv_a.reshape(KQ, 128, KV_LORA + QK_ROPE)
    wkval_t = bf(wkva_r[:, :, :KV_LORA].reshape(KQ, 128, 4, 128)
                 .transpose(1, 0, 2, 3))                          # [p,k,j,d]
    wkvap_t = bf(wkva_r[:, :, KV_LORA:][:, :, _PERM].transpose(1, 0, 2))
    # absorb weights [d, h, c] -> chunks of 4 heads: [4, 128, 4h, 4cc, 128]
    wkvbn_t = bf(wkv_b_r[:, :QK_NOPE, :].transpose(1, 0, 2)       # [d, h, c]
                 .reshape(128, 4, 4, 4, 128).transpose(1, 0, 2, 3, 4))
    # v-proj weights [c, h, d] -> [c_in_chunk, h, cc, d] -> 4-head chunks
    wkvbv_t = bf(wkv_b_r[:, QK_NOPE:, :].transpose(2, 0, 1)
                 .reshape(4, 128, N_HEADS, V_DIM).transpose(1, 2, 0, 3)
                 .reshape(128, 4, 4, 4, V_DIM).transpose(1, 0, 2, 3, 4))
    # [n, kk, p, j, cc]: wo_t[n,kk,p,j,cc] = wo[(kk*4+j)*128+p, n*512+cc]
    wo_t = bf(wo_f.reshape(4, 4, 128, 4, 512).transpose(3, 0, 2, 1, 4))

    qnwT = f32(q_norm_w).reshape(KB, 128, 1).transpose(1, 0, 2)
    kvnwT = f32(kv_norm_w).reshape(4, 128, 1).transpose(1, 0, 2)
    normw = c(np.concatenate([qnwT, kvnwT], axis=1))              # [128, 16, 1]

    # --- rope tables packed [32, 544]: cosq(256) sinq(256) cosk(16) sink(16)
    cos = f32(freqs_cos); sin = f32(freqs_sin)                    # [s=4, 32]
    cosT = np.tile(cos.T, (1, BPC))                               # [32, 16] m=(b,s)
    sinT = np.tile(sin.T, (1, BPC))
    cosq_t = np.repeat(cosT[:, None, :], N_HEADS, axis=1)         # [32, h, 16]
    sinq_t = np.repeat(sinT[:, None, :], N_HEADS, axis=1)
    ropet = c(np.concatenate(
        [cosq_t.reshape(R2, 256), sinq_t.reshape(R2, 256), cosT, sinT],
        axis=1))

    # --- fp8 caches in both layouts ---
    kv8 = np.asarray(kv_cache, dtype=np.float32).astype(NE3)      # [32, 8192, 512]
    pe8 = np.asarray(pe_cache, dtype=np.float32).astype(NE3)      # [32, 8192, 64]

    in_maps = []
    for ci in range(N_CORES):
        bs = slice(ci * BPC, (ci + 1) * BPC)
        kvb = kv8[bs]                                             # [4, 8192, 512]
        peb = pe8[bs]
        # scores-side: klat[b,j,p,t] = kv[b, t, j*128+p]
        klat_ = c(kvb.reshape(BPC, MAX_SEQ, 4, 128).transpose(0, 2, 3, 1))
        # peS[b,r,t] = pe[b, t, perm(r)]
        peS_ = c(peb[:, :, _PERM].transpose(0, 2, 1))
        # pv-side: kvnP[b,gg,p,i,c] = kv[b, gg*2048 + i*128 + p, c]
        kvnP_ = c(kvb.reshape(BPC, 4, 16, 128, KV_LORA).transpose(0, 1, 3, 2, 4))
        xc = bf(x[bs].reshape(M, DIM).T.reshape(KQ, 128, M).transpose(1, 0, 2))
        in_maps.append({
            "xT": xc,
            "wqa": wqa_t, "wqbn": wqbn_t, "wqbp": wqbp_t,
            "wkval": wkval_t, "wkvap": wkvap_t,
            "wkvbn": wkvbn_t, "wkvbv": wkvbv_t, "wo": wo_t,
            "normw": normw, "ropet": ropet,
            "klat": klat_, "peS": peS_, "kvnP": kvnP_,
        })
    return in_maps


def kernel(**inputs):
    in_maps = prep_in_maps(**inputs)

    if "nc" not in _NC_CACHE:
        _NC_CACHE["nc"] = build_bass()
    nc = _NC_CACHE["nc"]

    trace = os.environ.get("KERNEL_TRACE", "0") == "1"
    tmpdir = os.environ.get("KERNEL_TMPDIR") or None
    res = run_bass_kernel_spmd(
        nc, in_maps, core_ids=list(range(N_CORES)), trace=trace, tmpdir=tmpdir
    )
    if trace and res.exec_time_ns is not None:
        print(f"HW exec time: {res.exec_time_ns} ns")
        _NC_CACHE["last_exec_ns"] = res.exec_time_ns

    _NC_CACHE["res"] = res
    _NC_CACHE["results"] = res.results
    outs = [r["out"].reshape(BPC, SEQLEN, DIM) for r in res.results]
    return np.concatenate(outs, axis=0).astype(np.float32)


# revision 32
# speedup vs baseline: 1.0460x; 1.0282x over previous
"""MLA decode kernel for Trainium2, data-parallel over batch across 8 NeuronCores.

Each core handles 4 batches (M = 16 query rows).  v2 design vs baseline:
  - Attention matmuls stream the fp8 cache as the MOVING operand (512-col
    matmuls, ~144 per batch) with tiny Q/P stationaries, instead of pushing
    the cache through LDWEIGHTS as 128x128 stationary tiles (6400 instrs).
    Scores come out as S[hs, t]; softmax runs on the free axis (exp via
    ScalarE with fused row-sum accum_out).
  - P is transposed for PV with DVE StreamTranspose 32x32 blocks written at
    partition bases chosen per t-block, assembling a true [t%128, tc, hs]
    stationary layout in 8 instructions per batch.
  - PV output [hs, c] is PE-transposed (4 tiles) back to [c, hs] for the
    v-projection; v-proj / wo keep the baseline weight-stationary form.
  - Projections accumulate directly in PSUM across all k-chunks: one
    leading start=True per bank, then start=False everywhere - per-element
    has_written bits give overwrite-on-first-touch / accumulate-after,
    so many accumulation groups share a bank (validated on HW).  This
    removes the DVE accumulation chains that serialized the old q path.
  - kv/pe caches fp8 (e3m4) in both layouts; weights bf16.  Cache tiles are
    half-batch grained and prefetch through deep pools from t=0, spread
    over the sync/scalar/gpsimd DMA queues by need-time.
Host prep does layout/dtype only (transposes, tiling, fp8 cast) - no math.
"""

import os
import sys

sys.path.insert(0, "/opt/trn_rl_repo")

import numpy as np
import ml_dtypes

import concourse.bass as bass
import concourse.bacc as bacc_mod
import concourse.mybir as mybir
from concourse.bass_utils import run_bass_kernel_spmd
from concourse.masks import make_identity
from concourse.tile import TileContext

BF16 = mybir.dt.bfloat16
F32 = mybir.dt.float32
E3 = mybir.dt.float8e3
NBF = ml_dtypes.bfloat16
NE3 = ml_dtypes.float8_e3m4

DIM = 2048
N_HEADS = 16
Q_LORA = 1536
KV_LORA = 512
QK_NOPE = 128
QK_ROPE = 64
V_DIM = 128
QK_HD = QK_NOPE + QK_ROPE  # 192
MAX_SEQ = 8192
BSZ = 32
SEQLEN = 4
START_POS = MAX_SEQ - SEQLEN
EPS = 1e-6
SCALE = QK_HD ** -0.5

N_CORES = 8
BPC = BSZ // N_CORES          # batches per core = 4
M = BPC * SEQLEN              # rows per core = 16 (b, s)
HS = N_HEADS * SEQLEN         # 64 score rows per batch (h, s)
KQ = DIM // 128               # 16 k-chunks of x
KB = Q_LORA // 128            # 12 k-chunks of q_lora
R2 = QK_ROPE // 2             # 32
NTG = 16                      # t-groups of 512 per batch
TC = MAX_SEQ // 128           # 64 t-chunks of 128 per batch
HT = MAX_SEQ // 2             # 4096, half-tile width

AF = mybir.ActivationFunctionType
ALU = mybir.AluOpType


def build_bass():
    nc = bacc_mod.Bacc(target_bir_lowering=False)

    # ---- DRAM inputs (per core) ----
    xT = nc.dram_tensor("xT", [128, KQ, M], BF16, kind="ExternalInput")
    wqa = nc.dram_tensor("wqa", [KQ, 128, KB, 128], BF16, kind="ExternalInput")
    wqbn = nc.dram_tensor("wqbn", [KB, 128, N_HEADS, 128], BF16, kind="ExternalInput")
    wqbp = nc.dram_tensor("wqbp", [KB, 128, N_HEADS, QK_ROPE], BF16, kind="ExternalInput")
    wkval = nc.dram_tensor("wkval", [128, KQ, 4, 128], BF16, kind="ExternalInput")
    wkvap = nc.dram_tensor("wkvap", [128, KQ, QK_ROPE], BF16, kind="ExternalInput")
    wkvbn = nc.dram_tensor("wkvbn", [4, 128, 4, 4, 128], BF16, kind="ExternalInput")
    wkvbv = nc.dram_tensor("wkvbv", [4, 128, 4, 4, 128], BF16, kind="ExternalInput")
    wo = nc.dram_tensor("wo", [4, 4, 128, 4, 512], BF16, kind="ExternalInput")
    normw = nc.dram_tensor("normw", [128, KB + 4, 1], F32, kind="ExternalInput")
    ropet = nc.dram_tensor("ropet", [R2, 544], F32, kind="ExternalInput")
    klat = nc.dram_tensor("klat", [BPC, 4, 128, MAX_SEQ], E3, kind="ExternalInput")
    peS = nc.dram_tensor("peS", [BPC, QK_ROPE, MAX_SEQ], E3, kind="ExternalInput")
    kvnP = nc.dram_tensor("kvnP", [BPC, 4, 128, 16, 512], E3, kind="ExternalInput")
    out = nc.dram_tensor("out", [M, DIM], F32, kind="ExternalOutput")

    import contextlib

    with TileContext(nc) as tc, contextlib.ExitStack() as _ctx:
            cpool = _ctx.enter_context(tc.tile_pool(name="const", bufs=1))
            apool = _ctx.enter_context(tc.tile_pool(name="acts", bufs=1))
            wqa_pool = _ctx.enter_context(tc.tile_pool(name="wqa_s", bufs=3))
            wkval_pool = _ctx.enter_context(tc.tile_pool(name="wkval_s", bufs=2))
            wqb_pool = _ctx.enter_context(tc.tile_pool(name="wqb_s", bufs=3))
            wqbp_pool = _ctx.enter_context(tc.tile_pool(name="wqbp_s", bufs=3))
            wkvbn_pool = _ctx.enter_context(tc.tile_pool(name="wkvbn_s", bufs=4))
            kl_pool = _ctx.enter_context(tc.tile_pool(name="kl_s", bufs=5))
            pe_pool = _ctx.enter_context(tc.tile_pool(name="pe_s", bufs=2))
            kv_pool = _ctx.enter_context(tc.tile_pool(name="kv_s", bufs=5))
            pall_pool = _ctx.enter_context(tc.tile_pool(name="pall_s", bufs=2))
            pt_pool = _ctx.enter_context(tc.tile_pool(name="pt_s", bufs=2))
            fin_pool = _ctx.enter_context(tc.tile_pool(name="fin_s", bufs=1))
            ps_proj = _ctx.enter_context(tc.tile_pool(name="ps_proj", bufs=2, space="PSUM"))
            ps_small = _ctx.enter_context(tc.tile_pool(name="ps_small", bufs=1, space="PSUM"))
            ps_sc = _ctx.enter_context(tc.tile_pool(name="ps_sc", bufs=4, space="PSUM"))
            ps_pv = _ctx.enter_context(tc.tile_pool(name="ps_pv", bufs=1, space="PSUM"))
            # ---------- constants (gpsimd engine ops first) ----------
            ident = cpool.tile([128, 128], BF16)
            make_identity(nc, ident)
            ones_cf = cpool.tile([128, 1], F32)
            nc.gpsimd.memset(ones_cf, 1.0)
            ones_rf = cpool.tile([1, 128], F32)
            nc.gpsimd.memset(ones_rf, 1.0)
            eps1 = cpool.tile([1, 1], F32)
            nc.gpsimd.memset(eps1, EPS)

            # ---------- cache tiles b0 (gpsimd queue, from t=0) ----------
            # kl_tiles[b][j][half]: [128, 4096]; pe_tiles[b][half]: [64, 4096]
            kl_tiles = {}
            pe_tiles = {}

            def req_cache_tiles(b, eng):
                kl_tiles[b] = []
                for j in range(4):
                    t = kl_pool.tile([128, MAX_SEQ], E3, tag="kl")
                    eng.dma_start(t, klat[b, j])
                    kl_tiles[b].append(t)
                t = pe_pool.tile([QK_ROPE, MAX_SEQ], E3, tag="pe")
                eng.dma_start(t, peS[b])
                pe_tiles[b] = t

            # ---------- small residents (merged DMAs, sync queue) ----------
            xT_sb = cpool.tile([128, KQ, M], BF16)
            nc.sync.dma_start(xT_sb, xT[:, :, :])
            norm_sb = cpool.tile([128, KB + 4, 1], F32)
            nc.sync.dma_start(norm_sb, normw[:, :, :])
            qnwT_sb = norm_sb[:, 0:KB, :]
            kvnwT_sb = norm_sb[:, KB:KB + 4, :]
            rope_sb = cpool.tile([R2, 544], F32)
            nc.sync.dma_start(rope_sb, ropet[:, :])
            cosq_sb = rope_sb[:, 0:256].rearrange("p (h m) -> p h m", h=N_HEADS)
            sinq_sb = rope_sb[:, 256:512].rearrange("p (h m) -> p h m", h=N_HEADS)
            cosk_sb = rope_sb[:, 512:528]
            sink_sb = rope_sb[:, 528:544]
            wkvap_sb = cpool.tile([128, KQ, QK_ROPE], BF16)
            nc.sync.dma_start(wkvap_sb, wkvap[:, :, :])
            # wkval on gpsimd ahead of the b0 cache tiles
            wkval_sb = []
            for cnk in range(4):
                t = wkval_pool.tile([128, 4, 4, 128], BF16, tag="wkval")
                nc.gpsimd.dma_start(t, wkval[:, cnk * 4:(cnk + 1) * 4, :, :])
                wkval_sb.append(t)

            # wkvbn early (gpsimd) - absorb needs it by ~55us
            wkvbn_sb = []
            for cnk in range(4):
                t = wkvbn_pool.tile([128, 4, 4, 128], BF16, tag="wkvbn")
                nc.gpsimd.dma_start(t, wkvbn[cnk])
                wkvbn_sb.append(t)

            req_cache_tiles(0, nc.gpsimd)

            # ---------- q1 = (x @ wq_a)^T accumulated in PSUM ----------
            psq = ps_proj.tile([128, KB, M], F32, tag="pp")
            for k in range(KQ):
                wqa_sb = wqa_pool.tile([128, KB, 128], BF16, tag="wqa")
                (nc.scalar if k % 2 == 0 else nc.sync).dma_start(
                    wqa_sb, wqa[k, :, :, :])
                for ncol in range(KB):
                    nc.tensor.matmul(psq[:, ncol, :], wqa_sb[:, ncol, :],
                                     xT_sb[:, k, :],
                                     start=(k == 0 and ncol == 0),
                                     stop=(k == KQ - 1 and ncol == KB - 1),
                                     skip_group_check=True)

            # ---------- kv path in PSUM ----------
            psk = ps_proj.tile([128, 5, M], F32, tag="pp")
            first = True
            for k in range(KQ):
                for j in range(4):
                    nc.tensor.matmul(psk[:, j, :],
                                     wkval_sb[k // 4][:, k % 4, j, :],
                                     xT_sb[:, k, :],
                                     start=first, stop=False,
                                     skip_group_check=True)
                    first = False
            for k in range(KQ):
                nc.tensor.matmul(psk[0:QK_ROPE, 4, :], wkvap_sb[:, k, :],
                                 xT_sb[:, k, :],
                                 start=False, stop=(k == KQ - 1),
                                 skip_group_check=True)

            def rms_factor(ps_in, nchunks, nfeat, tag):
                """Per-column 1/rms broadcast [128, M] f32 from transposed psum."""
                sq = apool.tile([128, nchunks, M], F32, tag=f"rms_sq{tag}")
                nc.scalar.activation(out=sq, in_=ps_in, func=AF.Square)
                msq = ps_small.tile([1, nchunks * M], F32, tag="ps")
                nc.tensor.matmul(msq, ones_cf,
                                 sq.rearrange("p a m -> p (a m)"),
                                 start=True, stop=True)
                msq_sb = apool.tile([1, nchunks * M], F32, tag=f"rms_msq{tag}")
                nc.vector.tensor_copy(msq_sb, msq)
                msq_v = msq_sb.rearrange("p (a m) -> p a m", a=nchunks)
                prev = msq_v[:, 0, :]
                for a in range(1, nchunks):
                    red = apool.tile([1, M], F32, tag=f"rms_red{tag}{a % 2}")
                    nc.vector.tensor_tensor(red, prev, msq_v[:, a, :], op=ALU.add)
                    prev = red
                rstd = apool.tile([1, M], F32, tag=f"rms_rstd{tag}")
                nc.scalar.activation(
                    out=rstd, in_=prev,
                    func=AF.Sqrt, scale=1.0 / nfeat, bias=eps1)
                rinv = apool.tile([1, M], F32, tag=f"rms_rinv{tag}")
                nc.vector.reciprocal(rinv, rstd)
                bc_ps = ps_small.tile([128, M], F32, tag="ps")
                nc.tensor.matmul(bc_ps, ones_rf, rinv, start=True, stop=True)
                bc = apool.tile([128, M], F32, tag=f"rms_bc{tag}")
                nc.vector.tensor_copy(bc, bc_ps)
                return bc

            # kv norm + casts
            bckv = rms_factor(psk[:, 0:4, :], 4, KV_LORA, "kv")
            kvlatT8 = apool.tile([128, 4, M], E3)
            kvlatT_bf = apool.tile([128, 4, M], BF16)
            for j in range(4):
                nc.vector.scalar_tensor_tensor(
                    out=kvlatT_bf[:, j, :], in0=psk[:, j, :],
                    scalar=kvnwT_sb[:, j, :], in1=bckv,
                    op0=ALU.mult, op1=ALU.mult)
            nc.vector.tensor_copy(kvlatT8, kvlatT_bf)
            # natural-layout fp8 copy for the PV-side insert
            kvlat8 = apool.tile([M, KV_LORA], E3)
            for j in range(4):
                ptb = ps_small.tile([M, 128], BF16, tag="ps")
                nc.tensor.transpose(ptb, kvlatT_bf[:, j, :], ident)
                nc.vector.tensor_copy(kvlat8[:, j * 128:(j + 1) * 128], ptb)

            # k_pe rope (even/odd-split rows already via host perm)
            kpeT8 = apool.tile([QK_ROPE, M], E3)
            t1k = apool.tile([R2, M], F32, tag="ropek1")
            t2k = apool.tile([R2, M], F32, tag="ropek2")
            nc.vector.tensor_tensor(t1k, psk[0:R2, 4, :], cosk_sb, op=ALU.mult)
            nc.vector.tensor_tensor(t2k, psk[R2:QK_ROPE, 4, :], sink_sb, op=ALU.mult)
            nc.vector.tensor_tensor(kpeT8[0:R2, :], t1k, t2k, op=ALU.subtract)
            t1k2 = apool.tile([R2, M], F32, tag="ropek1")
            t2k2 = apool.tile([R2, M], F32, tag="ropek2")
            nc.vector.tensor_tensor(t1k2, psk[0:R2, 4, :], sink_sb, op=ALU.mult)
            nc.vector.tensor_tensor(t2k2, psk[R2:QK_ROPE, 4, :], cosk_sb, op=ALU.mult)
            nc.vector.tensor_tensor(kpeT8[R2:QK_ROPE, :], t1k2, t2k2, op=ALU.add)

            # ---------- cache inserts (overwrite rows start_pos..) ----------
            def inserts_kl(b, eng):
                for j in range(4):
                    eng.dma_start(
                        kl_tiles[b][j][:, MAX_SEQ - SEQLEN:],
                        kvlatT8[:, j, b * SEQLEN:(b + 1) * SEQLEN])
                eng.dma_start(
                    pe_tiles[b][:, MAX_SEQ - SEQLEN:],
                    kpeT8[:, b * SEQLEN:(b + 1) * SEQLEN])

            inserts_kl(0, nc.gpsimd)

            # ---------- q1 rms + cast ----------
            bcq = rms_factor(psq, KB, Q_LORA, "q")
            q1nT = apool.tile([128, KB, M], BF16)
            for kc in range(KB):
                nc.vector.scalar_tensor_tensor(
                    out=q1nT[:, kc, :], in0=psq[:, kc, :],
                    scalar=qnwT_sb[:, kc, :], in1=bcq,
                    op0=ALU.mult, op1=ALU.mult)

            # ---------- q2: nope + pe in PSUM ----------
            psn = ps_proj.tile([128, N_HEADS, M], F32, tag="pp")
            psp = ps_proj.tile([128, N_HEADS, M], F32, tag="pp")
            fn = True
            fp = True
            for k in range(KB):
                wqbn_sb = wqb_pool.tile([128, N_HEADS, 128], BF16, tag="wqbn")
                (nc.scalar if k % 2 == 0 else nc.sync).dma_start(
                    wqbn_sb, wqbn[k, :, :, :])
                wqbp_sb = wqbp_pool.tile([128, N_HEADS, QK_ROPE], BF16, tag="wqbp")
                (nc.sync if k % 2 == 0 else nc.scalar).dma_start(
                    wqbp_sb, wqbp[k, :, :, :])
                for h in range(N_HEADS):
                    nc.tensor.matmul(psn[:, h, :], wqbn_sb[:, h, :],
                                     q1nT[:, k, :],
                                     start=fn,
                                     stop=(k == KB - 1 and h == N_HEADS - 1),
                                     skip_group_check=True)
                    fn = False
                for h in range(N_HEADS):
                    nc.tensor.matmul(psp[0:QK_ROPE, h, :], wqbp_sb[:, h, :],
                                     q1nT[:, k, :],
                                     start=fp,
                                     stop=(k == KB - 1 and h == N_HEADS - 1),
                                     skip_group_check=True)
                    fp = False

            # ---------- remaining cache tiles + inserts ----------
            # Queue choice avoids FIFO deadlocks: pool-gated DMA issues must
            # never sit ahead (same engine FIFO) of instructions attention
            # needs.  kvn b0 / wo n0-n1 on scalar are gate-free; gated kvn
            # b1 goes to sync (only kl b3/outs behind it), kvn b2/b3 and wo
            # n2-n3 to gpsimd (nothing early behind them).
            kvn_tiles = {}

            def req_kvn(b, eng):
                kvn_tiles[b] = []
                for gg in range(4):
                    t = kv_pool.tile([128, 16, 512], E3, tag="kvn")
                    eng.dma_start(t, kvnP[b, gg])
                    kvn_tiles[b].append(t)
                eng.dma_start(
                    kvn_tiles[b][3][128 - SEQLEN:, 15, :],
                    kvlat8[b * SEQLEN:(b + 1) * SEQLEN, :])

            req_cache_tiles(1, nc.sync)
            inserts_kl(1, nc.sync)
            req_kvn(0, nc.scalar)
            req_kvn(1, nc.gpsimd)
            req_cache_tiles(2, nc.gpsimd)
            inserts_kl(2, nc.gpsimd)
            req_kvn(2, nc.scalar)
            req_cache_tiles(3, nc.sync)
            inserts_kl(3, nc.sync)
            req_kvn(3, nc.gpsimd)

            # wkvbv through the kl ring: slots free as scores b2/b3 finish
            wkvbv_sb = []
            for cnk in range(4):
                t = kl_pool.tile([128, 4, 4, 128], BF16, tag="kl")
                nc.scalar.dma_start(t, wkvbv[cnk])
                wkvbv_sb.append(t)

            # wo through the kvn ring: slots free as PV consumes cache tiles,
            # so wo streams just-in-time during late attention
            wo_sb = {}
            for n in range(4):
                for kk in range(4):
                    wot = kv_pool.tile([128, 4, 512], BF16, tag="kvn")
                    eng = nc.scalar if n < 2 else nc.gpsimd
                    eng.dma_start(wot, wo[n, kk])
                    wo_sb[(n, kk)] = wot

            # ---------- q rope + absorb -> QT ----------
            QT = apool.tile([128, 5, BPC, HS], BF16)
            t1q = apool.tile([R2, N_HEADS, M], F32, tag="ropeq1")
            t2q = apool.tile([R2, N_HEADS, M], F32, tag="ropeq2")
            nc.vector.tensor_tensor(t1q, psp[0:R2, :, :], cosq_sb, op=ALU.mult)
            nc.vector.tensor_tensor(t2q, psp[R2:QK_ROPE, :, :], sinq_sb, op=ALU.mult)
            for b in range(BPC):
                nc.vector.tensor_tensor(
                    QT[0:R2, 4, b, :].rearrange("p (h s) -> p h s", h=N_HEADS),
                    t1q[:, :, b * SEQLEN:(b + 1) * SEQLEN],
                    t2q[:, :, b * SEQLEN:(b + 1) * SEQLEN],
                    op=ALU.subtract)
            t3q = apool.tile([R2, N_HEADS, M], F32, tag="ropeq1")
            t4q = apool.tile([R2, N_HEADS, M], F32, tag="ropeq2")
            nc.vector.tensor_tensor(t3q, psp[0:R2, :, :], sinq_sb, op=ALU.mult)
            nc.vector.tensor_tensor(t4q, psp[R2:QK_ROPE, :, :], cosq_sb, op=ALU.mult)
            for b in range(BPC):
                nc.vector.tensor_tensor(
                    QT[R2:QK_ROPE, 4, b, :].rearrange("p (h s) -> p h s", h=N_HEADS),
                    t3q[:, :, b * SEQLEN:(b + 1) * SEQLEN],
                    t4q[:, :, b * SEQLEN:(b + 1) * SEQLEN],
                    op=ALU.add)

            qnT_sb = apool.tile([128, N_HEADS, M], BF16)
            nc.vector.tensor_copy(qnT_sb, psn)

            # absorb: QT[:, cc, b, h*4+s] = sum_d wkvbn[h][d, c] * qnope[m, h, d]
            for h in range(N_HEADS):
                pa4 = ps_small.tile([128, 4, M], F32, tag="ps")
                for cc in range(4):
                    nc.tensor.matmul(pa4[:, cc, :],
                                     wkvbn_sb[h // 4][:, h % 4, cc, :],
                                     qnT_sb[:, h, :], start=(cc == 0), stop=(cc == 3))
                nc.vector.tensor_copy(
                    QT[:, 0:4, :, h * SEQLEN:(h + 1) * SEQLEN],
                    pa4.rearrange("p cc (b s) -> p cc b s", b=BPC))

            # ---------- attention ----------
            # Scores for batch b+1 are emitted BEFORE PV of batch b, so the
            # PE never stalls waiting for b's kvn tiles (they get a full
            # extra batch-window to arrive) and the DVE P-transposes of b
            # overlap the next batch's score matmuls.
            outT = apool.tile([128, 4, N_HEADS, M], BF16)
            PTs = {}
            rinvs = {}

            def scores_phase(b):
                P_all = pall_pool.tile([128, 4, TC // 2, 32], BF16, tag="pall")
                PT = pt_pool.tile([128, TC, HS], BF16, tag="pt")
                PTs[b] = PT
                ssum_t = apool.tile([HS, NTG], F32, tag=f"ssum{b % 2}")

                def scores_quarter(qq):
                    # j-outer within the quarter: 4 consecutive matmuls share
                    # one Q stationary (amortizes the ~53ns LDWEIGHTS bubble
                    # measured per MM), accumulating into 4 banks at once.
                    tgs = list(range(qq * 4, qq * 4 + 4))
                    sps = []
                    for _ti in range(4):
                        sp = ps_sc.tile([128, 512], F32, tag="sp")
                        sps.append(sp)
                    for j in range(4):
                        for ti, tg in enumerate(tgs):
                            nc.tensor.matmul(
                                sps[ti][0:HS, :], QT[:, j, b, :],
                                kl_tiles[b][j][:, tg * 512:(tg + 1) * 512],
                                start=(j == 0), stop=False,
                                skip_group_check=True)
                    for ti, tg in enumerate(tgs):
                        nc.tensor.matmul(
                            sps[ti][0:HS, :], QT[0:QK_ROPE, 4, b, :],
                            pe_tiles[b][:, tg * 512:(tg + 1) * 512],
                            start=False, stop=True,
                            skip_group_check=True)
                        pb_ = 64 * (tg // 8)
                        tl = 4 * (tg % 8)
                        nc.scalar.activation(
                            out=P_all[pb_:pb_ + HS, :, tl:tl + 4, :]
                                .rearrange("p r q bb -> p q r bb"),
                            in_=sps[ti][0:HS, :],
                            func=AF.Exp, scale=SCALE,
                            accum_out=ssum_t[:, tg:tg + 1])

                def transpose_quarter(qq):
                    half, sub = divmod(qq, 2)
                    for r in range(4):
                        for i in range(2):
                            pb_ = 64 * half + 32 * i
                            nc.vector.transpose(
                                PT[32 * r:32 * (r + 1), 16 * qq:16 * qq + 16,
                                   32 * i:32 * (i + 1)],
                                P_all[pb_:pb_ + 32, r,
                                      16 * sub:16 * sub + 16, :])

                for qq in range(4):
                    scores_quarter(qq)
                    transpose_quarter(qq)

                # row sums -> 1/sum (DVE)
                s8 = apool.tile([HS, 8], F32, tag=f"att_s8{b % 2}")
                nc.vector.tensor_tensor(s8, ssum_t[:, 0:8], ssum_t[:, 8:16],
                                        op=ALU.add)
                s4 = apool.tile([HS, 4], F32, tag=f"att_s4{b % 2}")
                nc.vector.tensor_tensor(s4, s8[:, 0:4], s8[:, 4:8], op=ALU.add)
                s2 = apool.tile([HS, 2], F32, tag=f"att_s2{b % 2}")
                nc.vector.tensor_tensor(s2, s4[:, 0:2], s4[:, 2:4], op=ALU.add)
                ssum = apool.tile([HS, 1], F32, tag=f"att_ssum{b % 2}")
                nc.vector.tensor_tensor(ssum, s2[:, 0:1], s2[:, 1:2], op=ALU.add)
                rinv = apool.tile([HS, 1], F32, tag=f"att_rinv{b % 2}")
                nc.vector.reciprocal(rinv, ssum)
                rinvs[b] = rinv

            def pv_phase(b):
                PT = PTs[b]
                po = ps_pv.tile([128, 512], F32, tag="po")
                for tci in range(TC):
                    nc.tensor.matmul(po[0:HS, :], PT[:, tci, :],
                                     kvn_tiles[b][tci // 16][:, tci % 16, :],
                                     start=(tci == 0), stop=(tci == TC - 1),
                                     skip_group_check=True)
                po_sb = apool.tile([HS, 512], BF16, tag=f"po_sb{b % 2}")
                nc.vector.tensor_scalar(out=po_sb, in0=po[0:HS, :],
                                        scalar1=rinvs[b],
                                        scalar2=None, op0=ALU.mult)
                psT = ps_small.tile([128, 4, HS], BF16, tag="ps")
                for cc in range(4):
                    nc.tensor.transpose(psT[:, cc, :],
                                        po_sb[:, cc * 128:(cc + 1) * 128],
                                        ident[0:HS, 0:HS])
                nc.vector.tensor_copy(
                    outT[:, :, :, b * SEQLEN:(b + 1) * SEQLEN],
                    psT.rearrange("p cc (h s) -> p cc h s", h=N_HEADS))

            for b in range(BPC):
                scores_phase(b)
                pv_phase(b)

            # ---------- v-proj: o2T[d, h, m] ----------
            o2T = apool.tile([128, N_HEADS, M], BF16)
            for h in range(N_HEADS):
                pv = ps_small.tile([128, M], F32, tag="ps")
                for cc in range(4):
                    nc.tensor.matmul(pv, wkvbv_sb[h // 4][:, h % 4, cc, :],
                                     outT[:, cc, h, :],
                                     start=(cc == 0), stop=(cc == 3))
                nc.vector.tensor_copy(o2T[:, h, :], pv)

            # ---------- final: out = o2 @ wo ----------
            for n in range(4):
                pf = ps_proj.tile([M, 512], F32, tag="pp")
                for kk in range(4):
                    for j in range(4):
                        k = kk * 4 + j
                        nc.tensor.matmul(pf, o2T[:, k, :], wo_sb[(n, kk)][:, j, :],
                                         start=(k == 0), stop=(k == KQ - 1))
                fin = fin_pool.tile([M, 512], F32, tag="fin")
                nc.vector.tensor_copy(fin, pf)
                nc.sync.dma_start(out[:, n * 512:(n + 1) * 512], fin)

    nc.compile()
    return nc


_NC_CACHE = {}

# even/odd split permutation for rope dims: rows 0..31 = even pairs, 32..63 = odd
_PERM = np.concatenate([np.arange(0, QK_ROPE, 2), np.arange(1, QK_ROPE, 2)])


def prep_in_maps(x, wq_a, q_norm_w, wq_b, wkv_a, kv_norm_w, wkv_b, wo,
                 kv_cache, pe_cache, freqs_cos, freqs_sin, start_pos):
    assert int(start_pos) == START_POS
    bf = lambda a: np.ascontiguousarray(np.asarray(a, dtype=np.float32), dtype=NBF)
    f32 = lambda a: np.ascontiguousarray(np.asarray(a), dtype=np.float32)
    c = np.ascontiguousarray

    x = f32(x)
    wq_a = f32(wq_a); wq_b = f32(wq_b); wkv_a = f32(wkv_a)
    wkv_b_r = f32(wkv_b).reshape(N_HEADS, QK_NOPE + V_DIM, KV_LORA)
    wo_f = f32(wo)

    # --- weights, transposed/tiled layouts (bf16) ---
    wqa_t = bf(wq_a.reshape(KQ, 128, KB, 128))                    # [k,p,nc,d]
    wqb_r = wq_b.reshape(KB, 128, N_HEADS, QK_HD)                 # [k,p,h,d]
    wqbn_t = bf(wqb_r[:, :, :, :QK_NOPE])
    wqbp_t = bf(wqb_r[:, :, :, QK_NOPE:][:, :, :, _PERM])         # rope perm
    wkva_r = wkv_a.reshape(KQ, 128, KV_LORA + QK_ROPE)
    wkval_t = bf(wkva_r[:, :, :KV_LORA].reshape(KQ, 128, 4, 128)
                 .transpose(1, 0, 2, 3))                          # [p,k,j,d]
    wkvap_t = bf(wkva_r[:, :, KV_LORA:][:, :, _PERM].transpose(1, 0, 2))
    # absorb weights [d, h, c] -> chunks of 4 heads: [4, 128, 4h, 4cc, 128]
    wkvbn_t = bf(wkv_b_r[:, :QK_NOPE, :].transpose(1, 0, 2)       # [d, h, c]
                 .reshape(128, 4, 4, 4, 128).transpose(1, 0, 2, 3, 4))
    # v-proj weights [c, h, d] -> [c_in_chunk, h, cc, d] -> 4-head chunks
    wkvbv_t = bf(wkv_b_r[:, QK_NOPE:, :].transpose(2, 0, 1)
                 .reshape(4, 128, N_HEADS, V_DIM).transpose(1, 2, 0, 3)
                 .reshape(128, 4, 4, 4, V_DIM).transpose(1, 0, 2, 3, 4))
    # [n, kk, p, j, cc]: wo_t[n,kk,p,j,cc] = wo[(kk*4+j)*128+p, n*512+cc]
    wo_t = bf(wo_f.reshape(4, 4, 128, 4, 512).transpose(3, 0, 2, 1, 4))

    qnwT = f32(q_norm_w).reshape(KB, 128, 1).transpose(1, 0, 2)
    kvnwT = f32(kv_norm_w).reshape(4, 128, 1).transpose(1, 0, 2)
    normw = c(np.concatenate([qnwT, kvnwT], axis=1))              # [128, 16, 1]

    # --- rope tables packed [32, 544]: cosq(256) sinq(256) cosk(16) sink(16)
    cos = f32(freqs_cos); sin = f32(freqs_sin)                    # [s=4, 32]
    cosT = np.tile(cos.T, (1, BPC))                               # [32, 16] m=(b,s)
    sinT = np.tile(sin.T, (1, BPC))
    cosq_t = np.repeat(cosT[:, None, :], N_HEADS, axis=1)         # [32, h, 16]
    sinq_t = np.repeat(sinT[:, None, :], N_HEADS, axis=1)
    ropet = c(np.concatenate(
        [cosq_t.reshape(R2, 256), sinq_t.reshape(R2, 256), cosT, sinT],
        axis=1))

    # --- fp8 caches in both layouts ---
    kv8 = np.asarray(kv_cache, dtype=np.float32).astype(NE3)      # [32, 8192, 512]
    pe8 = np.asarray(pe_cache, dtype=np.float32).astype(NE3)      # [32, 8192, 64]

    in_maps = []
    for ci in range(N_CORES):
        bs = slice(ci * BPC, (ci + 1) * BPC)
        kvb = kv8[bs]                                             # [4, 8192, 512]
        peb = pe8[bs]
        # scores-side: klat[b,j,p,t] = kv[b, t, j*128+p]
        klat_ = c(kvb.reshape(BPC, MAX_SEQ, 4, 128).transpose(0, 2, 3, 1))
        # peS[b,r,t] = pe[b, t, perm(r)]
        peS_ = c(peb[:, :, _PERM].transpose(0, 2, 1))
        # pv-side: kvnP[b,gg,p,i,c] = kv[b, gg*2048 + i*128 + p, c]
        kvnP_ = c(kvb.reshape(BPC, 4, 16, 128, KV_LORA).transpose(0, 1, 3, 2, 4))
        xc = bf(x[bs].reshape(M, DIM).T.reshape(KQ, 128, M).transpose(1, 0, 2))
        in_maps.append({
            "xT": xc,
            "wqa": wqa_t, "wqbn": wqbn_t, "wqbp": wqbp_t,
            "wkval": wkval_t, "wkvap": wkvap_t,
            "wkvbn": wkvbn_t, "wkvbv": wkvbv_t, "wo": wo_t,
            "normw": normw, "ropet": ropet,
            "klat": klat_, "peS": peS_, "kvnP": kvnP_,
        })
    return in_maps


def kernel(**inputs):
    in_maps = prep_in_maps(**inputs)

    if "nc" not in _NC_CACHE:
        _NC_CACHE["nc"] = build_bass()
    nc = _NC_CACHE["nc"]

    trace = os.environ.get("KERNEL_TRACE", "0") == "1"
    tmpdir = os.environ.get("KERNEL_TMPDIR") or None
    res = run_bass_kernel_spmd(
        nc, in_maps, core_ids=list(range(N_CORES)), trace=trace, tmpdir=tmpdir
    )
    if trace and res.exec_time_ns is not None:
        print(f"HW exec time: {res.exec_time_ns} ns")
        _NC_CACHE["last_exec_ns"] = res.exec_time_ns

    _NC_CACHE["res"] = res
    _NC_CACHE["results"] = res.results
    outs = [r["out"].reshape(BPC, SEQLEN, DIM) for r in res.results]
    return np.concatenate(outs, axis=0).astype(np.float32)
